# revision 1
# baseline (speedup 1.0000x reference)
"""Trainium2 Bass kernel: BatchNorm -> 2-layer LSTM (32 steps, constant layer-0
input) -> Linear, data-parallel over batch across 8 NeuronCores.

Layout strategy (per core, batch shard = 128 rows):
  - All matmuls are out[b, j] = lhsT.T @ rhs with lhsT = h^T chunks [128d, 128b]
    (stationary) and rhs = W^T chunks [128d, N] (moving), accumulating fp32 in
    PSUM over the 8 contraction chunks of d=1024.
  - BN stats are computed from a replicated z^T (d on partitions, batch on the
    free axis) with bn_stats/bn_aggr; the normalized zn^T is produced directly
    in the transposed layout the matmuls need.
  - Weights are bf16 (host-prepared W.T), gates accumulate fp32, cell state c
    stays fp32; h is bf16.
  - Gates are computed in 4 column-groups of 256 per layer so PSUM (8 banks)
    can double/triple buffer; h slices are transposed back with PE transposes.
  - Biases enter via K=1 matmuls against a ones row (b rows stored [16, 256]).
  - h1^T is DMA'd to an HBM scratch each step; the final linear is a deferred
    phase that runs after the recurrent weights are released.
"""

import os
import sys

sys.path.insert(0, "/opt/trn_rl_repo")

import numpy as np
import ml_dtypes

import concourse.bass as bass
import concourse.bacc as bacc
import concourse.tile as tile
import concourse.mybir as mybir
from concourse import bass_utils
from concourse.masks import make_identity

BF16 = mybir.dt.bfloat16
F32 = mybir.dt.float32
AF = mybir.ActivationFunctionType
ALU = mybir.AluOpType

B = 1024          # batch
D = 1024          # hidden = input size
H4 = 4 * D        # gate width
BAR = int(os.environ.get("KERNEL_NSTEPS", "32"))
NCORES = int(os.environ.get("KERNEL_NCORES", "8"))
BS = B // 8       # batch shard per core (always 1/8th of batch)
EPS = 1e-5
KC = D // 128     # contraction chunks (8)
NG = 4            # gate column groups per layer
GS = D // NG      # group size in hidden cols (256)


def _np_bf16(a):
    return np.ascontiguousarray(a).astype(ml_dtypes.bfloat16)


def build_module(nsteps=BAR):
    nc = bacc.Bacc(
        "TRN2",
        target_bir_lowering=False,
        debug=False,
        enable_asserts=False,
        num_devices=NCORES,
        dynamic_dma_scratch_size=512,
    )

    # ---- DRAM I/O -------------------------------------------------------
    d_zT = nc.dram_tensor("zT", [D, B], F32, kind="ExternalInput").ap()
    d_zTs = nc.dram_tensor("zTs", [D, BS], F32, kind="ExternalInput").ap()
    d_gamma = nc.dram_tensor("gamma", [D, 1], F32, kind="ExternalInput").ap()
    d_beta = nc.dram_tensor("beta", [D, 1], F32, kind="ExternalInput").ap()
    d_wih0 = nc.dram_tensor("wt_ih0", [D, H4], BF16, kind="ExternalInput").ap()
    d_whh0 = nc.dram_tensor("wt_hh0", [D, H4], BF16, kind="ExternalInput").ap()
    d_wih1 = nc.dram_tensor("wt_ih1", [D, H4], BF16, kind="ExternalInput").ap()
    d_whh1 = nc.dram_tensor("wt_hh1", [D, H4], BF16, kind="ExternalInput").ap()
    d_wlin = nc.dram_tensor("wt_lin", [D, D], BF16, kind="ExternalInput").ap()
    d_b0 = nc.dram_tensor("b0t", [1, 16, 256], BF16, kind="ExternalInput").ap()
    d_b1bc = nc.dram_tensor("b1bc", [4, 128, 4, 256], BF16, kind="ExternalInput").ap()
    d_blin = nc.dram_tensor("b_lin", [D], F32, kind="ExternalInput").ap()
    d_out = nc.dram_tensor("out", [BS, nsteps, D], F32, kind="ExternalOutput").ap()
    d_h1sc = nc.dram_tensor("h1scratch", [nsteps, 128, KC, 128], BF16,
                            kind="Internal").ap()
    dbg = {}
    if os.environ.get("KERNEL_DEBUG"):
        dbg["znT"] = nc.dram_tensor("dbg_znT", [128, KC, 128], F32,
                                    kind="ExternalOutput").ap()
        dbg["c0i"] = nc.dram_tensor("dbg_c0i", [128, D], F32,
                                    kind="ExternalOutput").ap()
        dbg["x0"] = nc.dram_tensor("dbg_x0", [4, 128, 4, 256], BF16,
                                   kind="ExternalOutput").ap()
        dbg["h0T1"] = nc.dram_tensor("dbg_h0T1", [128, KC, 128], BF16,
                                     kind="ExternalOutput").ap()
        dbg["c01"] = nc.dram_tensor("dbg_c01", [128, D], F32,
                                    kind="ExternalOutput").ap()
        dbg["h1T1"] = nc.dram_tensor("dbg_h1T1", [128, KC, 128], BF16,
                                     kind="ExternalOutput").ap()
        dbg["c11"] = nc.dram_tensor("dbg_c11", [128, D], F32,
                                    kind="ExternalOutput").ap()
    d_x0 = nc.dram_tensor("x0scratch", [4, 128, 4, 256], BF16,
                          kind="Internal").ap()

    with tile.TileContext(nc) as tc:
        build_body(nc, tc, nsteps,
                   d_zT, d_zTs, d_gamma, d_beta,
                   d_wih0, d_whh0, d_wih1, d_whh1, d_wlin,
                   d_b0, d_b1bc, d_blin, d_out, d_h1sc, d_x0, dbg)
    nc.compile()
    return nc


def build_body(nc, tc, nsteps, d_zT, d_zTs, d_gamma, d_beta,
               d_wih0, d_whh0, d_wih1, d_whh1, d_wlin,
               d_b0, d_b1bc, d_blin, d_out, d_h1sc, d_x0, dbg):
    # ---- whole-life SBUF ------------------------------------------------
    life = tc.alloc_tile_pool(name="life", bufs=1)
    whh0 = life.tile([128, KC, H4], BF16, tag="whh0")
    wih1 = life.tile([128, KC, H4], BF16, tag="wih1")
    c0 = life.tile([128, D], F32, tag="c0")
    c1 = life.tile([128, D], F32, tag="c1")
    h0T = life.tile([128, KC, 128], BF16, tag="h0T")
    ones = life.tile([1, 128], BF16, tag="ones")
    idbf = life.tile([128, 128], BF16, tag="idbf")

    h1Tp = tc.alloc_tile_pool(name="h1Tp", bufs=2)

    nc.vector.memset(ones, 1.0)
    make_identity(nc, idbf)
    for k in range(KC):
        nc.sync.dma_start(out=whh0[:, k, :], in_=d_whh0[k * 128:(k + 1) * 128, :])
    for k in range(KC):
        nc.sync.dma_start(out=wih1[:, k, :], in_=d_wih1[k * 128:(k + 1) * 128, :])

    # ---- INIT phase: BN stats + zn^T + c0/c1 + x0_proj ------------------
    with tc.tile_pool(name="initp", bufs=2) as initp, \
         tc.tile_pool(name="small", bufs=4) as small, \
         tc.tile_pool(name="znp", bufs=1) as znp, \
         tc.tile_pool(name="ipsum", bufs=2, space="PSUM") as ipsum, \
         tc.tile_pool(name="tpsum", bufs=2, space="PSUM") as tpsum:

        eps_t = small.tile([128, 1], F32, tag="eps")
        nc.vector.memset(eps_t, EPS)
        znf = znp.tile([128, KC, 128], F32, tag="znf")
        idf32 = znp.tile([128, 128], F32, tag="idf32")
        make_identity(nc, idf32)

        for k in range(KC):
            zt = initp.tile([128, B], F32, tag="zt")
            nc.sync.dma_start(out=zt, in_=d_zT[k * 128:(k + 1) * 128, :])
            zs = small.tile([128, BS], F32, tag="zs")
            nc.sync.dma_start(out=zs, in_=d_zTs[k * 128:(k + 1) * 128, :])
            gk = small.tile([128, 1], F32, tag="gk")
            nc.sync.dma_start(out=gk, in_=d_gamma[k * 128:(k + 1) * 128, :])
            bk = small.tile([128, 1], F32, tag="bk")
            nc.sync.dma_start(out=bk, in_=d_beta[k * 128:(k + 1) * 128, :])

            st = small.tile([128, 2, 6], F32, tag="st")
            nc.vector.bn_stats(out=st[:, 0, :], in_=zt[:, 0:512])
            nc.vector.bn_stats(out=st[:, 1, :], in_=zt[:, 512:1024])
            mv = small.tile([128, 2], F32, tag="mv")
            nc.vector.bn_aggr(out=mv, in_=st)

            sd = small.tile([128, 1], F32, tag="sd")
            nc.scalar.activation(out=sd, in_=mv[:, 1:2], func=AF.Sqrt, bias=eps_t)
            rs = small.tile([128, 1], F32, tag="rs")
            nc.vector.reciprocal(out=rs, in_=sd)
            sc = small.tile([128, 1], F32, tag="sc")
            nc.vector.tensor_mul(sc, gk, rs)

            # zn^T chunk (fp32): (z - mean) * scale + beta
            nc.vector.tensor_scalar(
                out=znf[:, k, :], in0=zs,
                scalar1=mv[:, 0:1], scalar2=sc,
                op0=ALU.subtract, op1=ALU.mult)
            nc.vector.tensor_scalar_add(znf[:, k, :], znf[:, k, :], bk)
            # bf16 copy for matmul lhsT (h0 initial state)
            nc.vector.tensor_copy(out=h0T[:, k, :], in_=znf[:, k, :])

        if dbg:
            nc.sync.dma_start(out=dbg["znT"], in_=znf)
        # c0 = c1 = zn in [b, d] layout via PE transpose of fp32 zn^T
        for k in range(KC):
            pt = tpsum.tile([128, 128], F32, tag="tpz")
            nc.tensor.transpose(pt, znf[:, k, :], idf32)
            nc.scalar.copy(out=c0[:, k * 128:(k + 1) * 128], in_=pt)
        nc.vector.tensor_copy(out=c1, in_=c0)
        if dbg:
            nc.sync.dma_start(out=dbg["c0i"], in_=c0)

        # x0_proj = zn @ W_ih0^T + (b_ih0 + b_hh0), group-major bf16.
        # Two halves of 2 groups each so PSUM fits (W_ih0 chunks DMA'd twice).
        b0t = znp.tile([1, 16, 256], BF16, tag="b0t")
        nc.sync.dma_start(out=b0t, in_=d_b0)
        for half in range(2):
            gs_ = (0, 1) if half == 0 else (2, 3)
            psg = {g: ipsum.tile([128, 4, GS], F32, tag="ips", name=f"ips_{half}_{g}") for g in gs_}
            for g in gs_:
                for q in range(4):
                    nc.tensor.matmul(psg[g][:, q, :], ones,
                                     b0t[:, 4 * g + q, :],
                                     start=(q in (0, 2)), stop=False,
                                     skip_group_check=True)
            for k in range(KC):
                wk = initp.tile([128, H4], BF16, tag="wi0")
                nc.sync.dma_start(out=wk, in_=d_wih0[k * 128:(k + 1) * 128, :])
                for g in gs_:
                    for q in range(4):
                        nc.tensor.matmul(
                            psg[g][:, q, :], h0T[:, k, :],
                            wk[:, q * D + g * GS:q * D + (g + 1) * GS],
                            start=False, stop=(k == KC - 1),
                            skip_group_check=True)
            for g in gs_:
                xs = initp.tile([128, 4, GS], BF16, tag="xs", name=f"xs_{g}")
                nc.scalar.copy(out=xs, in_=psg[g])
                nc.sync.dma_start(out=d_x0[g], in_=xs)
                if dbg:
                    nc.sync.dma_start(out=dbg["x0"][g], in_=xs)

    # ---- W_hh1 load (after init pools release their SBUF) ---------------
    wlife = tc.alloc_tile_pool(name="wlife", bufs=1)
    whh1 = wlife.tile([128, KC, H4], BF16, tag="whh1")
    for k in range(KC):
        nc.sync.dma_start(out=whh1[:, k, :], in_=d_whh1[k * 128:(k + 1) * 128, :])

    # ---- recurrent loop --------------------------------------------------
    with tc.tile_pool(name="gates", bufs=3, space="PSUM") as gpool, \
         tc.tile_pool(name="trp", bufs=2, space="PSUM") as trpool, \
         tc.tile_pool(name="tmp", bufs=3) as tmp, \
         tc.tile_pool(name="xst", bufs=3) as xst, \
         tc.tile_pool(name="hst", bufs=4) as hst:

        h1T = h1Tp.tile([128, KC, 128], BF16, tag="h1T")
        nc.vector.tensor_copy(out=h1T, in_=h0T)

        def cell_math(g, ps, c, x0_slice, hs):
            # gates: ps[:, 0..3, :] = i, f, g, o (pre-activation, fp32 psum)
            if x0_slice is not None:
                nc.vector.tensor_add(ps, ps, x0_slice)
            nc.scalar.activation(out=ps[:, 0:2, :], in_=ps[:, 0:2, :], func=AF.Sigmoid)
            tg = tmp.tile([128, GS], F32, tag="tg")
            nc.scalar.activation(out=tg, in_=ps[:, 2, :], func=AF.Tanh)
            nc.scalar.activation(out=ps[:, 3, :], in_=ps[:, 3, :], func=AF.Sigmoid)
            csl = c[:, g * GS:(g + 1) * GS]
            nc.vector.tensor_mul(csl, csl, ps[:, 1, :])          # c *= sig(f)
            tp = tmp.tile([128, GS], F32, tag="tp")
            nc.vector.tensor_mul(tp, ps[:, 0, :], tg)            # sig(i)*tanh(g)
            nc.vector.tensor_add(csl, csl, tp)
            tc2 = tmp.tile([128, GS], F32, tag="tp")
            nc.scalar.activation(out=tc2, in_=csl, func=AF.Tanh)
            nc.vector.tensor_mul(hs, ps[:, 3, :], tc2)           # h = sig(o)*tanh(c)

        def transpose_pair(g, hs, hT):
            pt = trpool.tile([128, 256], BF16, tag="tr")
            nc.tensor.transpose(pt[:, 0:128], hs[:, 0:128], idbf)
            nc.tensor.transpose(pt[:, 128:256], hs[:, 128:256], idbf)
            nc.scalar.copy(out=hT[:, 2 * g:2 * g + 2, :], in_=pt)

        for t in range(nsteps):
            # --- layer 0: gates0 = x0_proj + h0 @ W_hh0^T ---
            ps0 = []
            hs0 = []
            for g in range(NG):
                xt = xst.tile([128, 4, GS], BF16, tag="x", name=f"x0_{t}_{g}")
                nc.sync.dma_start(out=xt, in_=d_x0[g])
                ps = gpool.tile([128, 4, GS], F32, tag="g")
                ps0.append(ps)
                for k in range(KC):
                    for q in range(4):
                        nc.tensor.matmul(
                            ps[:, q, :], h0T[:, k, :],
                            whh0[:, k, q * D + g * GS:q * D + (g + 1) * GS],
                            start=(k == 0 and q in (0, 2)),
                            stop=(k == KC - 1),
                            skip_group_check=True)
                hs = hst.tile([128, GS], BF16, tag="h")
                hs0.append(hs)
                cell_math(g, ps, c0, xt, hs)

            # --- layer 1 in two half-batches of groups to bound PSUM use ---
            h1T_new = h1Tp.tile([128, KC, 128], BF16, tag="h1T")
            for half in range(2):
                gs_ = (0, 1) if half == 0 else (2, 3)
                ps1 = {}
                for g in gs_:
                    ps = gpool.tile([128, 4, GS], F32, tag="g")
                    ps1[g] = ps
                    for k in range(KC):
                        for q in range(4):
                            nc.tensor.matmul(
                                ps[:, q, :], h1T[:, k, :],
                                whh1[:, k, q * D + g * GS:q * D + (g + 1) * GS],
                                start=(k == 0 and q in (0, 2)), stop=False,
                                skip_group_check=True)
                if half == 0:
                    # transpose h0 slices now (L0 math has had time to finish)
                    for g in range(NG):
                        transpose_pair(g, hs0[g], h0T)
                for g in gs_:
                    ps = ps1[g]
                    for k in range(KC):
                        for q in range(4):
                            nc.tensor.matmul(
                                ps[:, q, :], h0T[:, k, :],
                                wih1[:, k, q * D + g * GS:q * D + (g + 1) * GS],
                                start=False,
                                stop=(k == KC - 1),
                                skip_group_check=True)
                    bt = xst.tile([128, 4, GS], BF16, tag="x", name=f"b1_{t}_{g}")
                    nc.sync.dma_start(out=bt, in_=d_b1bc[g])
                    hs = hst.tile([128, GS], BF16, tag="h")
                    cell_math(g, ps, c1, bt, hs)
                    transpose_pair(g, hs, h1T_new)

            nc.sync.dma_start(out=d_h1sc[t], in_=h1T_new)
            if dbg and t == 0:
                nc.sync.dma_start(out=dbg["h0T1"], in_=h0T)
                nc.sync.dma_start(out=dbg["c01"], in_=c0)
                nc.sync.dma_start(out=dbg["h1T1"], in_=h1T_new)
                nc.sync.dma_start(out=dbg["c11"], in_=c1)
            h1T = h1T_new

    # Release recurrent weights/state so the final phase can use their SBUF.
    wlife.release()
    h1Tp.release()
    life.release()

    # ---- final linear: y_t = h1_t @ W_lin^T + b_lin ---------------------
    with tc.tile_pool(name="finw", bufs=1) as finw, \
         tc.tile_pool(name="fin", bufs=3) as fin, \
         tc.tile_pool(name="ypsum", bufs=3, space="PSUM") as ypool:

        wlin = finw.tile([128, KC, D], BF16, tag="wlin")
        for k in range(KC):
            nc.sync.dma_start(out=wlin[:, k, :], in_=d_wlin[k * 128:(k + 1) * 128, :])
        blin = finw.tile([128, D], F32, tag="blin")
        blin_b = bass.AP(tensor=d_blin.tensor, offset=d_blin.offset,
                         ap=[[0, 128], [1, D]])
        nc.sync.dma_start(out=blin, in_=blin_b)

        for t in range(nsteps):
            h1in = fin.tile([128, KC, 128], BF16, tag="h1in")
            nc.sync.dma_start(out=h1in, in_=d_h1sc[t])
            yp = ypool.tile([128, D], F32, tag="y")
            for k in range(KC):
                for n in range(2):
                    nc.tensor.matmul(
                        yp[:, n * 512:(n + 1) * 512], h1in[:, k, :],
                        wlin[:, k, n * 512:(n + 1) * 512],
                        start=(k == 0), stop=(k == KC - 1),
                        skip_group_check=True)
            ys = fin.tile([128, D], F32, tag="ys")
            nc.vector.tensor_add(ys, yp, blin)
            nc.sync.dma_start(out=d_out[:, t, :], in_=ys)


_CACHE = {}


def _get_module(nsteps=BAR):
    if nsteps not in _CACHE:
        _CACHE[nsteps] = build_module(nsteps)
    return _CACHE[nsteps]


def prep_inputs(z, bn_gamma, bn_beta, W_ih0, W_hh0, b_ih0, b_hh0,
                W_ih1, W_hh1, b_ih1, b_hh1, W_lin, b_lin):
    z = np.asarray(z, np.float32)
    zT = np.ascontiguousarray(z.T)
    common = {
        "zT": zT,
        "gamma": np.asarray(bn_gamma, np.float32).reshape(D, 1),
        "beta": np.asarray(bn_beta, np.float32).reshape(D, 1),
        "wt_ih0": _np_bf16(np.asarray(W_ih0, np.float32).T),
        "wt_hh0": _np_bf16(np.asarray(W_hh0, np.float32).T),
        "wt_ih1": _np_bf16(np.asarray(W_ih1, np.float32).T),
        "wt_hh1": _np_bf16(np.asarray(W_hh1, np.float32).T),
        "wt_lin": _np_bf16(np.asarray(W_lin, np.float32).T),
        "b0t": _np_bf16((np.asarray(b_ih0, np.float32)
                         + np.asarray(b_hh0, np.float32))
                        .reshape(4, 4, 256).transpose(1, 0, 2).reshape(1, 16, 256)),
        "b1bc": _np_bf16(np.broadcast_to(
            (np.asarray(b_ih1, np.float32) + np.asarray(b_hh1, np.float32))
            .reshape(4, 1, 4, 256).transpose(2, 1, 0, 3), (4, 128, 4, 256))),
        "b_lin": np.asarray(b_lin, np.float32),
    }
    in_maps = []
    for c in range(NCORES):
        m = dict(common)
        m["zTs"] = np.ascontiguousarray(zT[:, c * BS:(c + 1) * BS])
        in_maps.append(m)
    return in_maps


def kernel(**inputs):
    nc = _get_module()
    in_maps = prep_inputs(**inputs)
    res = bass_utils.run_bass_kernel_spmd(nc, in_maps, core_ids=list(range(NCORES)))
    out = np.concatenate([res.results[c]["out"] for c in range(NCORES)], axis=0)
    return out.astype(np.float32)



# revision 8
# speedup vs baseline: 1.0205x; 1.0205x over previous
"""Trainium2 Bass kernel: BatchNorm -> 2-layer LSTM (32 steps, constant layer-0
input) -> Linear, data-parallel over batch across 8 NeuronCores.

v2 layout strategy (per core, batch shard = 128 rows):
  - All matmuls: out[b, j] = lhsT.T @ rhs with lhsT = h^T chunks [128d, 128b]
    (stationary, fp16) and rhs = W^T chunks [128d, N] (moving, fp16),
    fp32 PSUM accumulation over 8 contraction chunks of d=1024.
  - Weights are host-packed fp16 in GROUP-MAJOR column order with gate
    order (i, f, o, g): col = g*1024 + q*256 + j.  Each gates matmul is
    then a contiguous [128, 512] rhs slice (one PSUM bank per instr,
    2 instrs per (group, k-chunk)) -- 2x fewer PE instructions.
  - x0_proj (+b0) and the L1 bias are RESIDENT in SBUF: zero per-step
    HBM traffic except the h1^T scratch store for the deferred linear.
  - wih0 (init-only) is staged through whh1's SBUF slot chunk-by-chunk;
    whh1's real load is issued right behind it.  b0t aliases x0r's rows.
  - Gate activations run in-place on PSUM (sigmoid over the contiguous
    i,f,o block, one instr), cell state c kept in fp16.
"""

import os
import sys

sys.path.insert(0, "/opt/trn_rl_repo")

import numpy as np

import concourse.bass as bass
import concourse.bacc as bacc
import concourse.tile as tile
import concourse.mybir as mybir
from concourse import bass_utils
from concourse.masks import make_identity

F16 = mybir.dt.float16
F32 = mybir.dt.float32
AF = mybir.ActivationFunctionType
ALU = mybir.AluOpType

B = 1024          # batch
D = 1024          # hidden = input size
H4 = 4 * D        # gate width
BAR = int(os.environ.get("KERNEL_NSTEPS", "32"))
NCORES = 8
BS = B // 8       # batch shard per core
EPS = 1e-5
KC = D // 128     # contraction chunks (8)
NG = 4            # gate column groups per layer
GS = D // NG      # group size in hidden cols (256)
PERM = (0, 1, 3, 2)  # gate order i,f,o,g (torch order is i,f,g,o)


def build_module(nsteps=BAR):
    nc = bacc.Bacc(
        "TRN2",
        target_bir_lowering=False,
        debug=False,
        enable_asserts=False,
        num_devices=NCORES,
        dynamic_dma_scratch_size=512,
    )

    # ---- DRAM I/O -------------------------------------------------------
    d_zT = nc.dram_tensor("zT", [D, B], F16, kind="ExternalInput").ap()
    d_zTs = nc.dram_tensor("zTs", [D, BS], F32, kind="ExternalInput").ap()
    d_gamma = nc.dram_tensor("gamma", [D, 1], F32, kind="ExternalInput").ap()
    d_beta = nc.dram_tensor("beta", [D, 1], F32, kind="ExternalInput").ap()
    d_wih0 = nc.dram_tensor("wt_ih0", [D, H4], F16, kind="ExternalInput").ap()
    d_whh0 = nc.dram_tensor("wt_hh0", [D, H4], F16, kind="ExternalInput").ap()
    d_wih1 = nc.dram_tensor("wt_ih1", [D, H4], F16, kind="ExternalInput").ap()
    d_whh1 = nc.dram_tensor("wt_hh1", [D, H4], F16, kind="ExternalInput").ap()
    d_wlin = nc.dram_tensor("wt_lin", [D, D], F16, kind="ExternalInput").ap()
    d_b0t = nc.dram_tensor("b0t", [1, 16, GS], F16, kind="ExternalInput").ap()
    d_b1v = nc.dram_tensor("b1v", [1, H4], F16, kind="ExternalInput").ap()
    d_blin = nc.dram_tensor("b_lin", [D], F32, kind="ExternalInput").ap()
    d_out = nc.dram_tensor("out", [BS, nsteps, D], F32, kind="ExternalOutput").ap()
    d_h1sc = nc.dram_tensor("h1scratch", [nsteps, 128, KC, 128], F16,
                            kind="Internal").ap()

    with tile.TileContext(nc) as tc:
        build_body(nc, tc, nsteps,
                   d_zT, d_zTs, d_gamma, d_beta,
                   d_wih0, d_whh0, d_wih1, d_whh1, d_wlin,
                   d_b0t, d_b1v, d_blin, d_out, d_h1sc)
    nc.compile()
    return nc


def build_body(nc, tc, nsteps, d_zT, d_zTs, d_gamma, d_beta,
               d_wih0, d_whh0, d_wih1, d_whh1, d_wlin,
               d_b0t, d_b1v, d_blin, d_out, d_h1sc):
    # ---- whole-life SBUF (219.4 KB/partition) ---------------------------
    life = tc.alloc_tile_pool(name="life", bufs=1)
    whh0 = life.tile([128, KC, H4], F16, tag="whh0")
    wih1 = life.tile([128, KC, H4], F16, tag="wih1")
    whh1 = life.tile([128, KC, H4], F16, tag="whh1")
    x0r = life.tile([128, 16, GS], F16, tag="x0r")    # (g, q, 256) resident
    b1r = life.tile([128, 16, GS], F16, tag="b1r")    # (g, q, 256) resident
    c0 = life.tile([128, D], F16, tag="c0")
    c1 = life.tile([128, D], F16, tag="c1")
    h0T = life.tile([128, KC, 128], F16, tag="h0T")
    idf = life.tile([128, 128], F16, tag="idf")

    make_identity(nc, idf)
    # Recurrent weight loads (wih0 is staged through whh1's slot below).
    for k in range(KC):
        nc.sync.dma_start(out=whh0[:, k, :], in_=d_whh0[k * 128:(k + 1) * 128, :])
    for k in range(KC):
        nc.sync.dma_start(out=wih1[:, k, :], in_=d_wih1[k * 128:(k + 1) * 128, :])
    # b1 broadcast into resident tile (128 copies of the packed 4096-vec)
    b1bc = bass.AP(tensor=d_b1v.tensor, offset=d_b1v.offset,
                   ap=[[0, 128], [1, H4]])
    nc.sync.dma_start(out=b1r, in_=b1bc)

    # ---- INIT phase: BN -> zn^T (=h0T) ; x0_proj ; c0/c1 ----------------
    with tc.tile_pool(name="initp", bufs=4) as initp, \
         tc.tile_pool(name="small", bufs=4) as small, \
         tc.tile_pool(name="ipsum", bufs=4, space="PSUM") as ipsum:

        eps_t = small.tile([128, 1], F32, tag="eps")
        nc.vector.memset(eps_t, EPS)
        ones = small.tile([1, 128], F16, tag="ones")
        nc.vector.memset(ones, 1.0)
        # b0t aliases the first partition-row of x0r (read fully before
        # x0r's own writes; tile tracker orders the WAR hazard).
        b0t = x0r[0:1, :, :]
        nc.sync.dma_start(out=b0t, in_=d_b0t)

        # stage wih0 chunks in whh1's slot; real whh1 load chases each chunk
        for k in range(KC):
            nc.sync.dma_start(out=whh1[:, k, :],
                              in_=d_wih0[k * 128:(k + 1) * 128, :])

        # x0_proj psum: all four groups live at once (8 banks), bias first
        psg = [ipsum.tile([128, 4, GS], F32, tag="ips", name=f"ips{g}")
               for g in range(NG)]
        for g in range(NG):
            for half in range(2):
                nc.tensor.matmul(psg[g][:, 2 * half:2 * half + 2, :], ones,
                                 b0t[:, 4 * g + 2 * half:4 * g + 2 * half + 2, :],
                                 start=True, stop=False, skip_group_check=True)

        for k in range(KC):
            # BN stats for d-chunk k from the full batch (fp16 z^T)
            zt0 = initp.tile([128, B // 2], F16, tag="zt")
            zt1 = initp.tile([128, B // 2], F16, tag="zt")
            nc.sync.dma_start(out=zt0, in_=d_zT[k * 128:(k + 1) * 128, 0:512])
            nc.sync.dma_start(out=zt1, in_=d_zT[k * 128:(k + 1) * 128, 512:1024])
            zs = small.tile([128, BS], F32, tag="zs")
            nc.sync.dma_start(out=zs, in_=d_zTs[k * 128:(k + 1) * 128, :])
            gk = small.tile([128, 1], F32, tag="gk")
            nc.sync.dma_start(out=gk, in_=d_gamma[k * 128:(k + 1) * 128, :])
            bk = small.tile([128, 1], F32, tag="bk")
            nc.sync.dma_start(out=bk, in_=d_beta[k * 128:(k + 1) * 128, :])

            st = small.tile([128, 2, 6], F32, tag="st")
            nc.vector.bn_stats(out=st[:, 0, :], in_=zt0)
            nc.vector.bn_stats(out=st[:, 1, :], in_=zt1)
            mv = small.tile([128, 2], F32, tag="mv")
            nc.vector.bn_aggr(out=mv, in_=st)

            sd = small.tile([128, 1], F32, tag="sd")
            nc.scalar.activation(out=sd, in_=mv[:, 1:2], func=AF.Sqrt, bias=eps_t)
            rs = small.tile([128, 1], F32, tag="rs")
            nc.vector.reciprocal(out=rs, in_=sd)
            sc = small.tile([128, 1], F32, tag="sc")
            nc.vector.tensor_mul(sc, gk, rs)

            # zn^T chunk straight into fp16 h0T: (z - mean) * scale + beta
            nc.vector.tensor_scalar(
                out=h0T[:, k, :], in0=zs,
                scalar1=mv[:, 0:1], scalar2=sc,
                op0=ALU.subtract, op1=ALU.mult)
            nc.vector.tensor_scalar_add(h0T[:, k, :], h0T[:, k, :], bk)

            # x0_proj contribution of chunk k (wih0 staged in whh1 slot)
            for g in range(NG):
                for half in range(2):
                    nc.tensor.matmul(
                        psg[g][:, 2 * half:2 * half + 2, :], h0T[:, k, :],
                        whh1[:, k, g * D + half * 512:g * D + (half + 1) * 512],
                        start=False, stop=(k == KC - 1),
                        skip_group_check=True)
            # chase with the real whh1 chunk load (WAR-ordered by tile)
            nc.sync.dma_start(out=whh1[:, k, :],
                              in_=d_whh1[k * 128:(k + 1) * 128, :])

        # psum -> resident x0 (fp16)
        for g in range(NG):
            nc.scalar.copy(out=x0r[:, 4 * g:4 * g + 4, :], in_=psg[g])

        # c0 = c1 = zn via PE transpose of fp16 zn^T (psum free again)
        for kk in range(KC // 2):
            pt = ipsum.tile([128, 2, 128], F16, tag="ips", name=f"tpz{kk}")
            nc.tensor.transpose(pt[:, 0, :], h0T[:, 2 * kk, :], idf)
            nc.tensor.transpose(pt[:, 1, :], h0T[:, 2 * kk + 1, :], idf)
            nc.scalar.copy(out=c0[:, kk * 256:(kk + 1) * 256], in_=pt)
        nc.vector.tensor_copy(out=c1, in_=c0)

    # ---- recurrent loop --------------------------------------------------
    h1Tp = tc.alloc_tile_pool(name="h1Tp", bufs=2)
    with tc.tile_pool(name="gates", bufs=3, space="PSUM") as gpool, \
         tc.tile_pool(name="trp", bufs=2, space="PSUM") as trpool, \
         tc.tile_pool(name="tmp", bufs=3) as tmp, \
         tc.tile_pool(name="hst", bufs=4) as hst:

        h1T = h1Tp.tile([128, KC, 128], F16, tag="h1T")
        nc.vector.tensor_copy(out=h1T, in_=h0T)

        def cell_math(g, ps, c, bias_src, hs):
            # ps[:, 0..3, :] = i, f, o, g preactivations (psum f32)
            nc.vector.tensor_add(ps, ps, bias_src[:, 4 * g:4 * g + 4, :])
            nc.scalar.activation(out=ps[:, 0:3, :], in_=ps[:, 0:3, :],
                                 func=AF.Sigmoid)
            tg = tmp.tile([128, GS], F16, tag="tp")
            nc.scalar.activation(out=tg, in_=ps[:, 3, :], func=AF.Tanh)
            csl = c[:, g * GS:(g + 1) * GS]
            nc.vector.tensor_mul(csl, csl, ps[:, 1, :])          # c *= sig(f)
            tp = tmp.tile([128, GS], F16, tag="tp")
            nc.vector.tensor_mul(tp, ps[:, 0, :], tg)            # sig(i)*tanh(g)
            nc.vector.tensor_add(csl, csl, tp)
            tc2 = tmp.tile([128, GS], F16, tag="tp")
            nc.scalar.activation(out=tc2, in_=csl, func=AF.Tanh)
            nc.vector.tensor_mul(hs, ps[:, 2, :], tc2)           # h = sig(o)*tanh(c)

        def transpose_pair(g, hs, hT):
            pt = trpool.tile([128, 256], F16, tag="tr")
            nc.tensor.transpose(pt[:, 0:128], hs[:, 0:128], idf)
            nc.tensor.transpose(pt[:, 128:256], hs[:, 128:256], idf)
            nc.scalar.copy(out=hT[:, 2 * g:2 * g + 2, :], in_=pt)

        def gates_matmul(ps, hT, w, g, first, last):
            # accumulate group g's 4 gate blocks: contiguous [128, 512] rhs
            for k in range(KC):
                for half in range(2):
                    nc.tensor.matmul(
                        ps[:, 2 * half:2 * half + 2, :], hT[:, k, :],
                        w[:, k, g * D + half * 512:g * D + (half + 1) * 512],
                        start=(first and k == 0),
                        stop=(last and k == KC - 1),
                        skip_group_check=True)

        for t in range(nsteps):
            # --- layer 0: gates0 = x0_proj + h0 @ W_hh0^T ---
            ps0 = []
            hs0 = []
            for g in range(NG):
                ps = gpool.tile([128, 4, GS], F32, tag="g")
                ps0.append(ps)
                gates_matmul(ps, h0T, whh0, g, True, True)
                hs = hst.tile([128, GS], F16, tag="h")
                hs0.append(hs)
                cell_math(g, ps, c0, x0r, hs)

            # --- layer 1 in two halves to bound PSUM use ---
            h1T_new = h1Tp.tile([128, KC, 128], F16, tag="h1T")
            for half in range(2):
                gs_ = (0, 1) if half == 0 else (2, 3)
                ps1 = {}
                for g in gs_:
                    ps = gpool.tile([128, 4, GS], F32, tag="g")
                    ps1[g] = ps
                    gates_matmul(ps, h1T, whh1, g, True, False)
                if half == 0:
                    # transpose h0 slices now (L0 math has had time to finish)
                    for g in range(NG):
                        transpose_pair(g, hs0[g], h0T)
                for g in gs_:
                    ps = ps1[g]
                    gates_matmul(ps, h0T, wih1, g, False, True)
                    hs = hst.tile([128, GS], F16, tag="h")
                    cell_math(g, ps, c1, b1r, hs)
                    transpose_pair(g, hs, h1T_new)

            nc.sync.dma_start(out=d_h1sc[t], in_=h1T_new)
            h1T = h1T_new

    # Release recurrent weights/state so the final phase can use their SBUF.
    h1Tp.release()
    life.release()

    # ---- final linear: y_t = h1_t @ W_lin^T + b_lin ---------------------
    with tc.tile_pool(name="finw", bufs=1) as finw, \
         tc.tile_pool(name="fin", bufs=3) as fin, \
         tc.tile_pool(name="ypsum", bufs=3, space="PSUM") as ypool:

        wlin = finw.tile([128, KC, D], F16, tag="wlin")
        for k in range(KC):
            nc.sync.dma_start(out=wlin[:, k, :], in_=d_wlin[k * 128:(k + 1) * 128, :])
        blin = finw.tile([128, D], F32, tag="blin")
        blin_b = bass.AP(tensor=d_blin.tensor, offset=d_blin.offset,
                         ap=[[0, 128], [1, D]])
        nc.sync.dma_start(out=blin, in_=blin_b)

        for t in range(nsteps):
            h1in = fin.tile([128, KC, 128], F16, tag="h1in")
            nc.sync.dma_start(out=h1in, in_=d_h1sc[t])
            yp = ypool.tile([128, D], F32, tag="y")
            for k in range(KC):
                for n in range(2):
                    nc.tensor.matmul(
                        yp[:, n * 512:(n + 1) * 512], h1in[:, k, :],
                        wlin[:, k, n * 512:(n + 1) * 512],
                        start=(k == 0), stop=(k == KC - 1),
                        skip_group_check=True)
            ys = fin.tile([128, D], F32, tag="ys")
            nc.vector.tensor_add(ys, yp, blin)
            nc.sync.dma_start(out=d_out[:, t, :], in_=ys)


_CACHE = {}


def _get_module(nsteps=BAR):
    if nsteps not in _CACHE:
        _CACHE[nsteps] = build_module(nsteps)
    return _CACHE[nsteps]


def _pack_w(W):
    """W [4D, D] torch gate rows -> [D, 4D] fp16, cols g*1024 + q'*256 + j
    with gate order q' = (i, f, o, g)."""
    W = np.asarray(W, np.float32)
    Wr = W.reshape(4, NG, GS, D)[list(PERM)]   # [q'][g][j][d]
    Wr = Wr.transpose(3, 1, 0, 2)              # [d][g][q'][j]
    return np.ascontiguousarray(Wr.reshape(D, H4)).astype(np.float16)


def _pack_b(b):
    b = np.asarray(b, np.float32)
    br = b.reshape(4, NG, GS)[list(PERM)]      # [q'][g][j]
    return np.ascontiguousarray(br.transpose(1, 0, 2))  # [g][q'][j]


def prep_inputs(z, bn_gamma, bn_beta, W_ih0, W_hh0, b_ih0, b_hh0,
                W_ih1, W_hh1, b_ih1, b_hh1, W_lin, b_lin):
    z = np.asarray(z, np.float32)
    zT = np.ascontiguousarray(z.T)
    b0 = _pack_b(np.asarray(b_ih0, np.float32) + np.asarray(b_hh0, np.float32))
    b1 = _pack_b(np.asarray(b_ih1, np.float32) + np.asarray(b_hh1, np.float32))
    common = {
        "zT": zT.astype(np.float16),
        "gamma": np.asarray(bn_gamma, np.float32).reshape(D, 1),
        "beta": np.asarray(bn_beta, np.float32).reshape(D, 1),
        "wt_ih0": _pack_w(W_ih0),
        "wt_hh0": _pack_w(W_hh0),
        "wt_ih1": _pack_w(W_ih1),
        "wt_hh1": _pack_w(W_hh1),
        "wt_lin": np.ascontiguousarray(np.asarray(W_lin, np.float32).T)
                    .astype(np.float16),
        "b0t": b0.reshape(1, 16, GS).astype(np.float16),
        "b1v": b1.reshape(1, H4).astype(np.float16),
        "b_lin": np.asarray(b_lin, np.float32),
    }
    in_maps = []
    for c in range(NCORES):
        m = dict(common)
        m["zTs"] = np.ascontiguousarray(zT[:, c * BS:(c + 1) * BS])
        in_maps.append(m)
    return in_maps


def kernel(**inputs):
    nc = _get_module()
    in_maps = prep_inputs(**inputs)
    res = bass_utils.run_bass_kernel_spmd(nc, in_maps, core_ids=list(range(NCORES)))
    out = np.concatenate([res.results[c]["out"] for c in range(NCORES)], axis=0)
    return out.astype(np.float32)


# revision 13
# speedup vs baseline: 1.0402x; 1.0194x over previous
"""Trainium2 Bass kernel: BatchNorm -> 2-layer LSTM (32 steps, constant layer-0
input) -> Linear, data-parallel over batch across 8 NeuronCores.

v2 layout strategy (per core, batch shard = 128 rows):
  - All matmuls: out[b, j] = lhsT.T @ rhs with lhsT = h^T chunks [128d, 128b]
    (stationary, fp16) and rhs = W^T chunks [128d, N] (moving, fp16),
    fp32 PSUM accumulation over 8 contraction chunks of d=1024.
  - Weights are host-packed fp16 in GROUP-MAJOR column order with gate
    order (i, f, o, g): col = g*1024 + q*256 + j.  Each gates matmul is
    then a contiguous [128, 512] rhs slice (one PSUM bank per instr,
    2 instrs per (group, k-chunk)) -- 2x fewer PE instructions.
  - x0_proj (+b0) and the L1 bias are RESIDENT in SBUF: zero per-step
    HBM traffic except the h1^T scratch store for the deferred linear.
  - wih0 (init-only) is staged through whh1's SBUF slot chunk-by-chunk;
    whh1's real load is issued right behind it.  b0t aliases x0r's rows.
  - Gate activations run in-place on PSUM (sigmoid over the contiguous
    i,f,o block, one instr), cell state c kept in fp16.
"""

import os
import sys

sys.path.insert(0, "/opt/trn_rl_repo")

import numpy as np

import concourse.bass as bass
import concourse.bacc as bacc
import concourse.tile as tile
import concourse.mybir as mybir
from concourse import bass_utils
from concourse.masks import make_identity

F16 = mybir.dt.float16
F32 = mybir.dt.float32
AF = mybir.ActivationFunctionType
ALU = mybir.AluOpType

B = 1024          # batch
D = 1024          # hidden = input size
H4 = 4 * D        # gate width
BAR = int(os.environ.get("KERNEL_NSTEPS", "32"))
NCORES = 8
BS = B // 8       # batch shard per core
EPS = 1e-5
KC = D // 128     # contraction chunks (8)
NG = 4            # gate column groups per layer
GS = D // NG      # group size in hidden cols (256)
PERM = (0, 1, 3, 2)  # gate order i,f,o,g (torch order is i,f,g,o)


def build_module(nsteps=BAR):
    nc = bacc.Bacc(
        "TRN2",
        target_bir_lowering=False,
        debug=False,
        enable_asserts=False,
        num_devices=NCORES,
        dynamic_dma_scratch_size=512,
    )

    # ---- DRAM I/O -------------------------------------------------------
    d_zT = nc.dram_tensor("zT", [D, B], F16, kind="ExternalInput").ap()
    d_zTs = nc.dram_tensor("zTs", [D, BS], F32, kind="ExternalInput").ap()
    d_gamma = nc.dram_tensor("gamma", [D, 1], F32, kind="ExternalInput").ap()
    d_beta = nc.dram_tensor("beta", [D, 1], F32, kind="ExternalInput").ap()
    d_wih0 = nc.dram_tensor("wt_ih0", [D, H4], F16, kind="ExternalInput").ap()
    d_whh0 = nc.dram_tensor("wt_hh0", [D, H4], F16, kind="ExternalInput").ap()
    d_wih1 = nc.dram_tensor("wt_ih1", [D, H4], F16, kind="ExternalInput").ap()
    d_whh1 = nc.dram_tensor("wt_hh1", [D, H4], F16, kind="ExternalInput").ap()
    d_wlin = nc.dram_tensor("wt_lin", [D, D], F16, kind="ExternalInput").ap()
    d_b0t = nc.dram_tensor("b0t", [1, 16, GS], F16, kind="ExternalInput").ap()
    d_b1v = nc.dram_tensor("b1v", [1, H4], F16, kind="ExternalInput").ap()
    d_blin = nc.dram_tensor("b_lin", [D], F32, kind="ExternalInput").ap()
    d_out = nc.dram_tensor("out", [BS, nsteps, D], F32, kind="ExternalOutput").ap()
    d_h1sc = nc.dram_tensor("h1scratch", [nsteps, 128, KC, 128], F16,
                            kind="Internal").ap()

    with tile.TileContext(nc) as tc:
        build_body(nc, tc, nsteps,
                   d_zT, d_zTs, d_gamma, d_beta,
                   d_wih0, d_whh0, d_wih1, d_whh1, d_wlin,
                   d_b0t, d_b1v, d_blin, d_out, d_h1sc)
    nc.compile()
    return nc


def build_body(nc, tc, nsteps, d_zT, d_zTs, d_gamma, d_beta,
               d_wih0, d_whh0, d_wih1, d_whh1, d_wlin,
               d_b0t, d_b1v, d_blin, d_out, d_h1sc):
    # ---- whole-life SBUF (219.4 KB/partition) ---------------------------
    life = tc.alloc_tile_pool(name="life", bufs=1)
    whh0 = life.tile([128, KC, H4], F16, tag="whh0")
    wih1 = life.tile([128, KC, H4], F16, tag="wih1")
    whh1 = life.tile([128, KC, H4], F16, tag="whh1")
    x0r = life.tile([128, 16, GS], F16, tag="x0r")    # (g, q, 256) resident
    b1r = life.tile([128, 16, GS], F16, tag="b1r")    # (g, q, 256) resident
    c0 = life.tile([128, D], F16, tag="c0")
    c1 = life.tile([128, D], F16, tag="c1")
    h0T = life.tile([128, KC, 128], F16, tag="h0T")
    idf = life.tile([128, 128], F16, tag="idf")

    make_identity(nc, idf)
    # Bulk weight loads ride the scalar-engine DGE queue so the
    # latency-critical z/BN path (sync queue) never queues behind them.
    for k in range(KC):
        nc.scalar.dma_start(out=whh0[:, k, :], in_=d_whh0[k * 128:(k + 1) * 128, :])
    # stage wih0 chunks in whh1's slot (scalar queue, behind whh0)
    for k in range(KC):
        nc.scalar.dma_start(out=whh1[:, k, :],
                            in_=d_wih0[k * 128:(k + 1) * 128, :])
    # b1 broadcast into resident tile (128 copies of the packed 4096-vec)
    b1bc = bass.AP(tensor=d_b1v.tensor, offset=d_b1v.offset,
                   ap=[[0, 128], [1, H4]])
    nc.scalar.dma_start(out=b1r, in_=b1bc)

    # ---- INIT phase: BN -> zn^T (=h0T) ; x0_proj ; c0/c1 ----------------
    with tc.tile_pool(name="initp", bufs=4) as initp, \
         tc.tile_pool(name="small", bufs=4) as small, \
         tc.tile_pool(name="ipsum", bufs=4, space="PSUM") as ipsum:

        eps_t = small.tile([128, 1], F32, tag="eps")
        nc.vector.memset(eps_t, EPS)
        ones = small.tile([1, 128], F16, tag="ones")
        nc.vector.memset(ones, 1.0)
        # b0t aliases the first partition-row of x0r (read fully before
        # x0r's own writes; tile tracker orders the WAR hazard).
        b0t = x0r[0:1, :, :]
        nc.sync.dma_start(out=b0t, in_=d_b0t)

        # x0_proj psum: all four groups live at once (8 banks), bias first
        psg = [ipsum.tile([128, 4, GS], F32, tag="ips", name=f"ips{g}")
               for g in range(NG)]
        for g in range(NG):
            for half in range(2):
                nc.tensor.matmul(psg[g][:, 2 * half:2 * half + 2, :], ones,
                                 b0t[:, 4 * g + 2 * half:4 * g + 2 * half + 2, :],
                                 start=True, stop=False, skip_group_check=True)

        for k in range(KC):
            # BN stats for d-chunk k from the full batch (fp16 z^T)
            zt0 = initp.tile([128, B // 2], F16, tag="zt")
            zt1 = initp.tile([128, B // 2], F16, tag="zt")
            nc.sync.dma_start(out=zt0, in_=d_zT[k * 128:(k + 1) * 128, 0:512])
            nc.sync.dma_start(out=zt1, in_=d_zT[k * 128:(k + 1) * 128, 512:1024])
            zs = small.tile([128, BS], F32, tag="zs")
            nc.sync.dma_start(out=zs, in_=d_zTs[k * 128:(k + 1) * 128, :])
            gk = small.tile([128, 1], F32, tag="gk")
            nc.sync.dma_start(out=gk, in_=d_gamma[k * 128:(k + 1) * 128, :])
            bk = small.tile([128, 1], F32, tag="bk")
            nc.sync.dma_start(out=bk, in_=d_beta[k * 128:(k + 1) * 128, :])

            st = small.tile([128, 2, 6], F32, tag="st")
            nc.vector.bn_stats(out=st[:, 0, :], in_=zt0)
            nc.vector.bn_stats(out=st[:, 1, :], in_=zt1)
            mv = small.tile([128, 2], F32, tag="mv")
            nc.vector.bn_aggr(out=mv, in_=st)

            sd = small.tile([128, 1], F32, tag="sd")
            nc.scalar.activation(out=sd, in_=mv[:, 1:2], func=AF.Sqrt, bias=eps_t)
            rs = small.tile([128, 1], F32, tag="rs")
            nc.vector.reciprocal(out=rs, in_=sd)
            sc = small.tile([128, 1], F32, tag="sc")
            nc.vector.tensor_mul(sc, gk, rs)

            # zn^T chunk straight into fp16 h0T: (z - mean) * scale + beta
            nc.vector.tensor_scalar(
                out=h0T[:, k, :], in0=zs,
                scalar1=mv[:, 0:1], scalar2=sc,
                op0=ALU.subtract, op1=ALU.mult)
            nc.vector.tensor_scalar_add(h0T[:, k, :], h0T[:, k, :], bk)

            # x0_proj contribution of chunk k (wih0 staged in whh1 slot)
            for g in range(NG):
                for half in range(2):
                    nc.tensor.matmul(
                        psg[g][:, 2 * half:2 * half + 2, :], h0T[:, k, :],
                        whh1[:, k, g * D + half * 512:g * D + (half + 1) * 512],
                        start=False, stop=(k == KC - 1),
                        skip_group_check=True)
            # wih1 chunk + real whh1 chunk chase the z-path on the sync queue
            nc.sync.dma_start(out=wih1[:, k, :],
                              in_=d_wih1[k * 128:(k + 1) * 128, :])
            nc.sync.dma_start(out=whh1[:, k, :],
                              in_=d_whh1[k * 128:(k + 1) * 128, :])

        # psum -> resident x0 (fp16)
        for g in range(NG):
            nc.scalar.copy(out=x0r[:, 4 * g:4 * g + 4, :], in_=psg[g])

        # c0 = c1 = zn via PE transpose of fp16 zn^T (psum free again)
        for kk in range(KC // 2):
            pt = ipsum.tile([128, 2, 128], F16, tag="ips", name=f"tpz{kk}")
            nc.tensor.transpose(pt[:, 0, :], h0T[:, 2 * kk, :], idf)
            nc.tensor.transpose(pt[:, 1, :], h0T[:, 2 * kk + 1, :], idf)
            nc.scalar.copy(out=c0[:, kk * 256:(kk + 1) * 256], in_=pt)
        nc.vector.tensor_copy(out=c1, in_=c0)

    # ---- recurrent loop --------------------------------------------------
    h1Tp = tc.alloc_tile_pool(name="h1Tp", bufs=2)
    with tc.tile_pool(name="gates", bufs=3, space="PSUM") as gpool, \
         tc.tile_pool(name="trp", bufs=2, space="PSUM") as trpool, \
         tc.tile_pool(name="tmp", bufs=3) as tmp, \
         tc.tile_pool(name="hst", bufs=4) as hst:

        h1T = h1Tp.tile([128, KC, 128], F16, tag="h1T")
        nc.vector.tensor_copy(out=h1T, in_=h0T)

        def cell_math(g, ps, c, bias_src, hs):
            # ps[:, 0..3, :] = i, f, o, g preactivations (psum f32)
            nc.vector.tensor_add(ps, ps, bias_src[:, 4 * g:4 * g + 4, :])
            nc.scalar.activation(out=ps[:, 0:3, :], in_=ps[:, 0:3, :],
                                 func=AF.Sigmoid)
            tg = tmp.tile([128, GS], F16, tag="tp")
            nc.scalar.activation(out=tg, in_=ps[:, 3, :], func=AF.Tanh)
            csl = c[:, g * GS:(g + 1) * GS]
            nc.vector.tensor_mul(csl, csl, ps[:, 1, :])          # c *= sig(f)
            tp = tmp.tile([128, GS], F16, tag="tp")
            nc.vector.tensor_mul(tp, ps[:, 0, :], tg)            # sig(i)*tanh(g)
            nc.vector.tensor_add(csl, csl, tp)
            tc2 = tmp.tile([128, GS], F16, tag="tp")
            nc.scalar.activation(out=tc2, in_=csl, func=AF.Tanh)
            nc.vector.tensor_mul(hs, ps[:, 2, :], tc2)           # h = sig(o)*tanh(c)

        def transpose_pair(g, hs, hT):
            pt = trpool.tile([128, 256], F16, tag="tr")
            nc.tensor.transpose(pt[:, 0:128], hs[:, 0:128], idf)
            nc.tensor.transpose(pt[:, 128:256], hs[:, 128:256], idf)
            nc.scalar.copy(out=hT[:, 2 * g:2 * g + 2, :], in_=pt)

        def gates_matmul(ps, hT, w, g, first, last):
            # accumulate group g's 4 gate blocks: contiguous [128, 512] rhs
            for k in range(KC):
                for half in range(2):
                    nc.tensor.matmul(
                        ps[:, 2 * half:2 * half + 2, :], hT[:, k, :],
                        w[:, k, g * D + half * 512:g * D + (half + 1) * 512],
                        start=(first and k == 0),
                        stop=(last and k == KC - 1),
                        skip_group_check=True)

        for t in range(nsteps):
            # --- layer 0: gates0 = x0_proj + h0 @ W_hh0^T ---
            ps0 = []
            hs0 = []
            for g in range(NG):
                ps = gpool.tile([128, 4, GS], F32, tag="g")
                ps0.append(ps)
                gates_matmul(ps, h0T, whh0, g, True, True)
                hs = hst.tile([128, GS], F16, tag="h")
                hs0.append(hs)
                cell_math(g, ps, c0, x0r, hs)

            # --- layer 1 in two halves to bound PSUM use ---
            h1T_new = h1Tp.tile([128, KC, 128], F16, tag="h1T")
            for half in range(2):
                gs_ = (0, 1) if half == 0 else (2, 3)
                ps1 = {}
                for g in gs_:
                    ps = gpool.tile([128, 4, GS], F32, tag="g")
                    ps1[g] = ps
                    gates_matmul(ps, h1T, whh1, g, True, False)
                if half == 0:
                    # transpose h0 slices now (L0 math has had time to finish)
                    for g in range(NG):
                        transpose_pair(g, hs0[g], h0T)
                for g in gs_:
                    ps = ps1[g]
                    gates_matmul(ps, h0T, wih1, g, False, True)
                    hs = hst.tile([128, GS], F16, tag="h")
                    cell_math(g, ps, c1, b1r, hs)
                    transpose_pair(g, hs, h1T_new)

            nc.sync.dma_start(out=d_h1sc[t], in_=h1T_new)
            h1T = h1T_new

    # Release recurrent weights/state so the final phase can use their SBUF.
    h1Tp.release()
    life.release()

    # ---- final linear: y_t = h1_t @ W_lin^T + b_lin ---------------------
    with tc.tile_pool(name="finw", bufs=1) as finw, \
         tc.tile_pool(name="fin", bufs=3) as fin, \
         tc.tile_pool(name="ypsum", bufs=3, space="PSUM") as ypool:

        wlin = finw.tile([128, KC, D], F16, tag="wlin")
        for k in range(KC):
            nc.scalar.dma_start(out=wlin[:, k, :],
                                in_=d_wlin[k * 128:(k + 1) * 128, :])
        blin = finw.tile([128, D], F32, tag="blin")
        blin_b = bass.AP(tensor=d_blin.tensor, offset=d_blin.offset,
                         ap=[[0, 128], [1, D]])
        nc.scalar.dma_start(out=blin, in_=blin_b)

        for t in range(nsteps):
            h1in = fin.tile([128, KC, 128], F16, tag="h1in")
            nc.sync.dma_start(out=h1in, in_=d_h1sc[t])
            yp = ypool.tile([128, D], F32, tag="y")
            for k in range(KC):
                for n in range(2):
                    nc.tensor.matmul(
                        yp[:, n * 512:(n + 1) * 512], h1in[:, k, :],
                        wlin[:, k, n * 512:(n + 1) * 512],
                        start=(k == 0), stop=(k == KC - 1),
                        skip_group_check=True)
            ys = fin.tile([128, D], F32, tag="ys")
            nc.vector.tensor_add(ys, yp, blin)
            nc.scalar.dma_start(out=d_out[:, t, :], in_=ys)


_CACHE = {}


def _get_module(nsteps=BAR):
    if nsteps not in _CACHE:
        _CACHE[nsteps] = build_module(nsteps)
    return _CACHE[nsteps]


def _pack_w(W):
    """W [4D, D] torch gate rows -> [D, 4D] fp16, cols g*1024 + q'*256 + j
    with gate order q' = (i, f, o, g)."""
    W = np.asarray(W, np.float32)
    Wr = W.reshape(4, NG, GS, D)[list(PERM)]   # [q'][g][j][d]
    Wr = Wr.transpose(3, 1, 0, 2)              # [d][g][q'][j]
    return np.ascontiguousarray(Wr.reshape(D, H4)).astype(np.float16)


def _pack_b(b):
    b = np.asarray(b, np.float32)
    br = b.reshape(4, NG, GS)[list(PERM)]      # [q'][g][j]
    return np.ascontiguousarray(br.transpose(1, 0, 2))  # [g][q'][j]


def prep_inputs(z, bn_gamma, bn_beta, W_ih0, W_hh0, b_ih0, b_hh0,
                W_ih1, W_hh1, b_ih1, b_hh1, W_lin, b_lin):
    z = np.asarray(z, np.float32)
    zT = np.ascontiguousarray(z.T)
    b0 = _pack_b(np.asarray(b_ih0, np.float32) + np.asarray(b_hh0, np.float32))
    b1 = _pack_b(np.asarray(b_ih1, np.float32) + np.asarray(b_hh1, np.float32))
    common = {
        "zT": zT.astype(np.float16),
        "gamma": np.asarray(bn_gamma, np.float32).reshape(D, 1),
        "beta": np.asarray(bn_beta, np.float32).reshape(D, 1),
        "wt_ih0": _pack_w(W_ih0),
        "wt_hh0": _pack_w(W_hh0),
        "wt_ih1": _pack_w(W_ih1),
        "wt_hh1": _pack_w(W_hh1),
        "wt_lin": np.ascontiguousarray(np.asarray(W_lin, np.float32).T)
                    .astype(np.float16),
        "b0t": b0.reshape(1, 16, GS).astype(np.float16),
        "b1v": b1.reshape(1, H4).astype(np.float16),
        "b_lin": np.asarray(b_lin, np.float32),
    }
    in_maps = []
    for c in range(NCORES):
        m = dict(common)
        m["zTs"] = np.ascontiguousarray(zT[:, c * BS:(c + 1) * BS])
        in_maps.append(m)
    return in_maps


def kernel(**inputs):
    nc = _get_module()
    in_maps = prep_inputs(**inputs)
    res = bass_utils.run_bass_kernel_spmd(nc, in_maps, core_ids=list(range(NCORES)))
    out = np.concatenate([res.results[c]["out"] for c in range(NCORES)], axis=0)
    return out.astype(np.float32)


# revision 18
# speedup vs baseline: 1.0428x; 1.0024x over previous
"""Trainium2 Bass kernel: BatchNorm -> 2-layer LSTM (32 steps, constant layer-0
input) -> Linear, data-parallel over batch across 8 NeuronCores.

v2 layout strategy (per core, batch shard = 128 rows):
  - All matmuls: out[b, j] = lhsT.T @ rhs with lhsT = h^T chunks [128d, 128b]
    (stationary, fp16) and rhs = W^T chunks [128d, N] (moving, fp16),
    fp32 PSUM accumulation over 8 contraction chunks of d=1024.
  - Weights are host-packed fp16 in GROUP-MAJOR column order with gate
    order (i, f, o, g): col = g*1024 + q*256 + j.  Each gates matmul is
    then a contiguous [128, 512] rhs slice (one PSUM bank per instr,
    2 instrs per (group, k-chunk)) -- 2x fewer PE instructions.
  - x0_proj (+b0) and the L1 bias are RESIDENT in SBUF: zero per-step
    HBM traffic except the h1^T scratch store for the deferred linear.
  - wih0 (init-only) is staged through whh1's SBUF slot chunk-by-chunk;
    whh1's real load is issued right behind it.  b0t aliases x0r's rows.
  - Gate activations run in-place on PSUM (sigmoid over the contiguous
    i,f,o block, one instr), cell state c kept in fp16.
"""

import os
import sys

sys.path.insert(0, "/opt/trn_rl_repo")

import numpy as np

import concourse.bass as bass
import concourse.bacc as bacc
import concourse.tile as tile
import concourse.mybir as mybir
from concourse import bass_utils
from concourse.masks import make_identity

F16 = mybir.dt.float16
F32 = mybir.dt.float32
AF = mybir.ActivationFunctionType
ALU = mybir.AluOpType

B = 1024          # batch
D = 1024          # hidden = input size
H4 = 4 * D        # gate width
BAR = int(os.environ.get("KERNEL_NSTEPS", "32"))
NCORES = 8
BS = B // 8       # batch shard per core
EPS = 1e-5
KC = D // 128     # contraction chunks (8)
NG = 4            # gate column groups per layer
GS = D // NG      # group size in hidden cols (256)
PERM = (0, 1, 3, 2)  # gate order i,f,o,g (torch order is i,f,g,o)


def build_module(nsteps=BAR):
    nc = bacc.Bacc(
        "TRN2",
        target_bir_lowering=False,
        debug=False,
        enable_asserts=False,
        num_devices=NCORES,
        dynamic_dma_scratch_size=512,
    )

    # ---- DRAM I/O -------------------------------------------------------
    d_zT = nc.dram_tensor("zT", [D, B], F16, kind="ExternalInput").ap()
    d_zTs = nc.dram_tensor("zTs", [D, BS], F32, kind="ExternalInput").ap()
    d_gamma = nc.dram_tensor("gamma", [D, 1], F32, kind="ExternalInput").ap()
    d_beta = nc.dram_tensor("beta", [D, 1], F32, kind="ExternalInput").ap()
    d_wih0 = nc.dram_tensor("wt_ih0", [D, H4], F16, kind="ExternalInput").ap()
    d_whh0 = nc.dram_tensor("wt_hh0", [D, H4], F16, kind="ExternalInput").ap()
    d_wih1 = nc.dram_tensor("wt_ih1", [D, H4], F16, kind="ExternalInput").ap()
    d_whh1 = nc.dram_tensor("wt_hh1", [D, H4], F16, kind="ExternalInput").ap()
    d_wlin = nc.dram_tensor("wt_lin", [D, D], F16, kind="ExternalInput").ap()
    d_b0t = nc.dram_tensor("b0t", [1, 16, GS], F16, kind="ExternalInput").ap()
    d_b1v = nc.dram_tensor("b1v", [1, H4], F16, kind="ExternalInput").ap()
    d_blin = nc.dram_tensor("b_lin", [D], F32, kind="ExternalInput").ap()
    d_out = nc.dram_tensor("out", [BS, nsteps, D], F32, kind="ExternalOutput").ap()
    d_h1sc = nc.dram_tensor("h1scratch", [nsteps, 128, KC, 128], F16,
                            kind="Internal").ap()

    with tile.TileContext(nc) as tc:
        build_body(nc, tc, nsteps,
                   d_zT, d_zTs, d_gamma, d_beta,
                   d_wih0, d_whh0, d_wih1, d_whh1, d_wlin,
                   d_b0t, d_b1v, d_blin, d_out, d_h1sc)
    nc.compile()
    return nc


def build_body(nc, tc, nsteps, d_zT, d_zTs, d_gamma, d_beta,
               d_wih0, d_whh0, d_wih1, d_whh1, d_wlin,
               d_b0t, d_b1v, d_blin, d_out, d_h1sc):
    # ---- whole-life SBUF (219.4 KB/partition) ---------------------------
    life = tc.alloc_tile_pool(name="life", bufs=1)
    whh0 = life.tile([128, KC, H4], F16, tag="whh0")
    wih1 = life.tile([128, KC, H4], F16, tag="wih1")
    whh1 = life.tile([128, KC, H4], F16, tag="whh1")
    x0r = life.tile([128, 16, GS], F16, tag="x0r")    # (g, q, 256) resident
    b1r = life.tile([128, 16, GS], F16, tag="b1r")    # (g, q, 256) resident
    c0 = life.tile([128, D], F16, tag="c0")
    c1 = life.tile([128, D], F16, tag="c1")
    h0T = life.tile([128, KC, 128], F16, tag="h0T")
    idf = life.tile([128, 128], F16, tag="idf")

    make_identity(nc, idf)
    # Weight arrival schedule (need-by order: staged wih0 ~asap, whh0 ~30us,
    # whh1 ~33us, wih1 35us+). Scalar DGE queue: staging + whh0 + even whh1
    # chases; sync DGE queue: z/BN path first, odd whh1 chases, then wih1.
    b1bc = bass.AP(tensor=d_b1v.tensor, offset=d_b1v.offset,
                   ap=[[0, 128], [1, H4]])
    nc.scalar.dma_start(out=b1r, in_=b1bc)
    # stage wih0 chunks in whh1's slot (x0_proj streams them immediately)
    for k in range(KC):
        nc.scalar.dma_start(out=whh1[:, k, :],
                            in_=d_wih0[k * 128:(k + 1) * 128, :])
    for k in range(KC):
        nc.scalar.dma_start(out=whh0[:, k, :], in_=d_whh0[k * 128:(k + 1) * 128, :])

    # ---- INIT phase: BN -> zn^T (=h0T) ; x0_proj ; c0/c1 ----------------
    with tc.tile_pool(name="initp", bufs=4) as initp, \
         tc.tile_pool(name="small", bufs=4) as small, \
         tc.tile_pool(name="ipsum", bufs=4, space="PSUM") as ipsum:

        eps_t = small.tile([128, 1], F32, tag="eps")
        nc.vector.memset(eps_t, EPS)
        ones = small.tile([1, 128], F16, tag="ones")
        nc.vector.memset(ones, 1.0)
        # b0t aliases the first partition-row of x0r (read fully before
        # x0r's own writes; tile tracker orders the WAR hazard).
        b0t = x0r[0:1, :, :]
        nc.sync.dma_start(out=b0t, in_=d_b0t)

        # x0_proj psum: all four groups live at once (8 banks), bias first
        psg = [ipsum.tile([128, 4, GS], F32, tag="ips", name=f"ips{g}")
               for g in range(NG)]
        for g in range(NG):
            for half in range(2):
                nc.tensor.matmul(psg[g][:, 2 * half:2 * half + 2, :], ones,
                                 b0t[:, 4 * g + 2 * half:4 * g + 2 * half + 2, :],
                                 start=True, stop=False, skip_group_check=True)

        for k in range(KC):
            # BN stats for d-chunk k from the full batch (fp16 z^T)
            zt0 = initp.tile([128, B // 2], F16, tag="zt")
            zt1 = initp.tile([128, B // 2], F16, tag="zt")
            nc.sync.dma_start(out=zt0, in_=d_zT[k * 128:(k + 1) * 128, 0:512])
            nc.sync.dma_start(out=zt1, in_=d_zT[k * 128:(k + 1) * 128, 512:1024])
            zs = small.tile([128, BS], F32, tag="zs")
            nc.sync.dma_start(out=zs, in_=d_zTs[k * 128:(k + 1) * 128, :])
            gk = small.tile([128, 1], F32, tag="gk")
            nc.sync.dma_start(out=gk, in_=d_gamma[k * 128:(k + 1) * 128, :])
            bk = small.tile([128, 1], F32, tag="bk")
            nc.sync.dma_start(out=bk, in_=d_beta[k * 128:(k + 1) * 128, :])

            st = small.tile([128, 2, 6], F32, tag="st")
            nc.vector.bn_stats(out=st[:, 0, :], in_=zt0)
            nc.vector.bn_stats(out=st[:, 1, :], in_=zt1)
            mv = small.tile([128, 2], F32, tag="mv")
            nc.vector.bn_aggr(out=mv, in_=st)

            sd = small.tile([128, 1], F32, tag="sd")
            nc.scalar.activation(out=sd, in_=mv[:, 1:2], func=AF.Sqrt, bias=eps_t)
            rs = small.tile([128, 1], F32, tag="rs")
            nc.vector.reciprocal(out=rs, in_=sd)
            sc = small.tile([128, 1], F32, tag="sc")
            nc.vector.tensor_mul(sc, gk, rs)

            # zn^T chunk straight into fp16 h0T: (z - mean) * scale + beta
            nc.vector.tensor_scalar(
                out=h0T[:, k, :], in0=zs,
                scalar1=mv[:, 0:1], scalar2=sc,
                op0=ALU.subtract, op1=ALU.mult)
            nc.vector.tensor_scalar_add(h0T[:, k, :], h0T[:, k, :], bk)

            # x0_proj contribution of chunk k (wih0 staged in whh1 slot)
            for g in range(NG):
                for half in range(2):
                    nc.tensor.matmul(
                        psg[g][:, 2 * half:2 * half + 2, :], h0T[:, k, :],
                        whh1[:, k, g * D + half * 512:g * D + (half + 1) * 512],
                        start=False, stop=(k == KC - 1),
                        skip_group_check=True)
            # real whh1 chunk chases the staged wih0 (WAR-ordered by tile);
            # alternate queues so neither backs up
            eng = nc.scalar if k % 2 == 0 else nc.sync
            eng.dma_start(out=whh1[:, k, :],
                          in_=d_whh1[k * 128:(k + 1) * 128, :])

        # wih1 is the last-needed weight: tail of the sync queue
        for k in range(KC):
            nc.sync.dma_start(out=wih1[:, k, :],
                              in_=d_wih1[k * 128:(k + 1) * 128, :])

        # psum -> resident x0 (fp16)
        for g in range(NG):
            nc.scalar.copy(out=x0r[:, 4 * g:4 * g + 4, :], in_=psg[g])

        # c0 = c1 = zn via PE transpose of fp16 zn^T (psum free again)
        for kk in range(KC // 2):
            pt = ipsum.tile([128, 2, 128], F16, tag="ips", name=f"tpz{kk}")
            nc.tensor.transpose(pt[:, 0, :], h0T[:, 2 * kk, :], idf)
            nc.tensor.transpose(pt[:, 1, :], h0T[:, 2 * kk + 1, :], idf)
            nc.scalar.copy(out=c0[:, kk * 256:(kk + 1) * 256], in_=pt)
        nc.vector.tensor_copy(out=c1, in_=c0)

    # ---- recurrent loop --------------------------------------------------
    h1Tp = tc.alloc_tile_pool(name="h1Tp", bufs=2)
    with tc.tile_pool(name="gates", bufs=3, space="PSUM") as gpool, \
         tc.tile_pool(name="trp", bufs=2, space="PSUM") as trpool, \
         tc.tile_pool(name="tmp", bufs=3) as tmp, \
         tc.tile_pool(name="hst", bufs=4) as hst:

        h1T = h1Tp.tile([128, KC, 128], F16, tag="h1T")
        nc.vector.tensor_copy(out=h1T, in_=h0T)

        def cell_math(g, ps, c, bias_src, hs):
            # ps[:, 0..3, :] = i, f, o, g preactivations (psum f32)
            nc.vector.tensor_add(ps, ps, bias_src[:, 4 * g:4 * g + 4, :])
            nc.scalar.activation(out=ps[:, 0:3, :], in_=ps[:, 0:3, :],
                                 func=AF.Sigmoid)
            tg = tmp.tile([128, GS], F16, tag="tp")
            nc.scalar.activation(out=tg, in_=ps[:, 3, :], func=AF.Tanh)
            csl = c[:, g * GS:(g + 1) * GS]
            nc.vector.tensor_mul(csl, csl, ps[:, 1, :])          # c *= sig(f)
            tp = tmp.tile([128, GS], F16, tag="tp")
            nc.vector.tensor_mul(tp, ps[:, 0, :], tg)            # sig(i)*tanh(g)
            nc.vector.tensor_add(csl, csl, tp)
            tc2 = tmp.tile([128, GS], F16, tag="tp")
            nc.scalar.activation(out=tc2, in_=csl, func=AF.Tanh)
            nc.vector.tensor_mul(hs, ps[:, 2, :], tc2)           # h = sig(o)*tanh(c)

        def transpose_pair(g, hs, hT):
            pt = trpool.tile([128, 256], F16, tag="tr")
            nc.tensor.transpose(pt[:, 0:128], hs[:, 0:128], idf)
            nc.tensor.transpose(pt[:, 128:256], hs[:, 128:256], idf)
            nc.scalar.copy(out=hT[:, 2 * g:2 * g + 2, :], in_=pt)

        def gates_matmul(ps, hT, w, g, first, last):
            # accumulate group g's 4 gate blocks: contiguous [128, 512] rhs
            for k in range(KC):
                for half in range(2):
                    nc.tensor.matmul(
                        ps[:, 2 * half:2 * half + 2, :], hT[:, k, :],
                        w[:, k, g * D + half * 512:g * D + (half + 1) * 512],
                        start=(first and k == 0),
                        stop=(last and k == KC - 1),
                        skip_group_check=True)

        for t in range(nsteps):
            # --- layer 0: gates0 = x0_proj + h0 @ W_hh0^T ---
            ps0 = []
            hs0 = []
            for g in range(NG):
                ps = gpool.tile([128, 4, GS], F32, tag="g")
                ps0.append(ps)
                gates_matmul(ps, h0T, whh0, g, True, True)
                hs = hst.tile([128, GS], F16, tag="h")
                hs0.append(hs)
                cell_math(g, ps, c0, x0r, hs)

            # --- layer 1 in two halves to bound PSUM use ---
            h1T_new = h1Tp.tile([128, KC, 128], F16, tag="h1T")
            for half in range(2):
                gs_ = (0, 1) if half == 0 else (2, 3)
                ps1 = {}
                for g in gs_:
                    ps = gpool.tile([128, 4, GS], F32, tag="g")
                    ps1[g] = ps
                    gates_matmul(ps, h1T, whh1, g, True, False)
                if half == 0:
                    # transpose h0 slices now (L0 math has had time to finish)
                    for g in range(NG):
                        transpose_pair(g, hs0[g], h0T)
                for g in gs_:
                    ps = ps1[g]
                    gates_matmul(ps, h0T, wih1, g, False, True)
                    hs = hst.tile([128, GS], F16, tag="h")
                    cell_math(g, ps, c1, b1r, hs)
                    transpose_pair(g, hs, h1T_new)

            nc.sync.dma_start(out=d_h1sc[t], in_=h1T_new)
            h1T = h1T_new

    # Release recurrent weights/state so the final phase can use their SBUF.
    h1Tp.release()
    life.release()

    # ---- final linear: y_t = h1_t @ W_lin^T + b_lin ---------------------
    with tc.tile_pool(name="finw", bufs=1) as finw, \
         tc.tile_pool(name="fin", bufs=3) as fin, \
         tc.tile_pool(name="ypsum", bufs=4, space="PSUM") as ypool:

        wlin = finw.tile([128, KC, D], F16, tag="wlin")
        for k in range(KC):
            nc.scalar.dma_start(out=wlin[:, k, :],
                                in_=d_wlin[k * 128:(k + 1) * 128, :])
        blin = finw.tile([128, D], F32, tag="blin")
        blin_b = bass.AP(tensor=d_blin.tensor, offset=d_blin.offset,
                         ap=[[0, 128], [1, D]])
        nc.scalar.dma_start(out=blin, in_=blin_b)

        for t in range(nsteps):
            h1in = fin.tile([128, KC, 128], F16, tag="h1in", bufs=4)
            nc.sync.dma_start(out=h1in, in_=d_h1sc[t])
            yp = ypool.tile([128, D], F32, tag="y")
            for k in range(KC):
                for n in range(2):
                    nc.tensor.matmul(
                        yp[:, n * 512:(n + 1) * 512], h1in[:, k, :],
                        wlin[:, k, n * 512:(n + 1) * 512],
                        start=(k == 0), stop=(k == KC - 1),
                        skip_group_check=True)
            ys = fin.tile([128, D], F32, tag="ys", bufs=6)
            nc.vector.tensor_add(ys, yp, blin)
            nc.scalar.dma_start(out=d_out[:, t, :], in_=ys)


_CACHE = {}


def _get_module(nsteps=BAR):
    if nsteps not in _CACHE:
        _CACHE[nsteps] = build_module(nsteps)
    return _CACHE[nsteps]


def _pack_w(W):
    """W [4D, D] torch gate rows -> [D, 4D] fp16, cols g*1024 + q'*256 + j
    with gate order q' = (i, f, o, g)."""
    W = np.asarray(W, np.float32)
    Wr = W.reshape(4, NG, GS, D)[list(PERM)]   # [q'][g][j][d]
    Wr = Wr.transpose(3, 1, 0, 2)              # [d][g][q'][j]
    return np.ascontiguousarray(Wr.reshape(D, H4)).astype(np.float16)


def _pack_b(b):
    b = np.asarray(b, np.float32)
    br = b.reshape(4, NG, GS)[list(PERM)]      # [q'][g][j]
    return np.ascontiguousarray(br.transpose(1, 0, 2))  # [g][q'][j]


def prep_inputs(z, bn_gamma, bn_beta, W_ih0, W_hh0, b_ih0, b_hh0,
                W_ih1, W_hh1, b_ih1, b_hh1, W_lin, b_lin):
    z = np.asarray(z, np.float32)
    zT = np.ascontiguousarray(z.T)
    b0 = _pack_b(np.asarray(b_ih0, np.float32) + np.asarray(b_hh0, np.float32))
    b1 = _pack_b(np.asarray(b_ih1, np.float32) + np.asarray(b_hh1, np.float32))
    common = {
        "zT": zT.astype(np.float16),
        "gamma": np.asarray(bn_gamma, np.float32).reshape(D, 1),
        "beta": np.asarray(bn_beta, np.float32).reshape(D, 1),
        "wt_ih0": _pack_w(W_ih0),
        "wt_hh0": _pack_w(W_hh0),
        "wt_ih1": _pack_w(W_ih1),
        "wt_hh1": _pack_w(W_hh1),
        "wt_lin": np.ascontiguousarray(np.asarray(W_lin, np.float32).T)
                    .astype(np.float16),
        "b0t": b0.reshape(1, 16, GS).astype(np.float16),
        "b1v": b1.reshape(1, H4).astype(np.float16),
        "b_lin": np.asarray(b_lin, np.float32),
    }
    in_maps = []
    for c in range(NCORES):
        m = dict(common)
        m["zTs"] = np.ascontiguousarray(zT[:, c * BS:(c + 1) * BS])
        in_maps.append(m)
    return in_maps


def kernel(**inputs):
    nc = _get_module()
    in_maps = prep_inputs(**inputs)
    res = bass_utils.run_bass_kernel_spmd(nc, in_maps, core_ids=list(range(NCORES)))
    out = np.concatenate([res.results[c]["out"] for c in range(NCORES)], axis=0)
    return out.astype(np.float32)


# revision 25
# speedup vs baseline: 1.0436x; 1.0008x over previous
"""Trainium2 Bass kernel: BatchNorm -> 2-layer LSTM (32 steps, constant layer-0
input) -> Linear, data-parallel over batch across 8 NeuronCores.

v2 layout strategy (per core, batch shard = 128 rows):
  - All matmuls: out[b, j] = lhsT.T @ rhs with lhsT = h^T chunks [128d, 128b]
    (stationary, fp16) and rhs = W^T chunks [128d, N] (moving, fp16),
    fp32 PSUM accumulation over 8 contraction chunks of d=1024.
  - Weights are host-packed fp16 in GROUP-MAJOR column order with gate
    order (i, f, o, g): col = g*1024 + q*256 + j.  Each gates matmul is
    then a contiguous [128, 512] rhs slice (one PSUM bank per instr,
    2 instrs per (group, k-chunk)) -- 2x fewer PE instructions.
  - x0_proj (+b0) and the L1 bias are RESIDENT in SBUF: zero per-step
    HBM traffic except the h1^T scratch store for the deferred linear.
  - wih0 (init-only) is staged through whh1's SBUF slot chunk-by-chunk;
    whh1's real load is issued right behind it.  b0t aliases x0r's rows.
  - Gate activations run in-place on PSUM (sigmoid over the contiguous
    i,f,o block, one instr), cell state c kept in fp16.
"""

import os
import sys

sys.path.insert(0, "/opt/trn_rl_repo")

import numpy as np

import concourse.bass as bass
import concourse.bacc as bacc
import concourse.tile as tile
import concourse.mybir as mybir
from concourse import bass_utils
from concourse.masks import make_identity

F16 = mybir.dt.float16
F32 = mybir.dt.float32
AF = mybir.ActivationFunctionType
ALU = mybir.AluOpType

B = 1024          # batch
D = 1024          # hidden = input size
H4 = 4 * D        # gate width
BAR = int(os.environ.get("KERNEL_NSTEPS", "32"))
NCORES = 8
BS = B // 8       # batch shard per core
EPS = 1e-5
KC = D // 128     # contraction chunks (8)
NG = 4            # gate column groups per layer
GS = D // NG      # group size in hidden cols (256)
PERM = (0, 1, 3, 2)  # gate order i,f,o,g (torch order is i,f,g,o)


def build_module(nsteps=BAR):
    nc = bacc.Bacc(
        "TRN2",
        target_bir_lowering=False,
        debug=False,
        enable_asserts=False,
        num_devices=NCORES,
        dynamic_dma_scratch_size=512,
    )

    # ---- DRAM I/O -------------------------------------------------------
    # Weights/z are host-packed partition-contiguous ([128, k*...]) so DMA
    # packets are 8-16KB instead of sub-KB (DMA is latency-bound per packet).
    d_zT = nc.dram_tensor("zT", [D, B], F16, kind="ExternalInput").ap()
    d_zTs = nc.dram_tensor("zTs", [128, KC * BS], F32, kind="ExternalInput").ap()
    d_gamma = nc.dram_tensor("gamma", [128, KC], F32, kind="ExternalInput").ap()
    d_beta = nc.dram_tensor("beta", [128, KC], F32, kind="ExternalInput").ap()
    d_wih0 = nc.dram_tensor("wt_ih0", [128, KC * H4], F16, kind="ExternalInput").ap()
    d_whh0 = nc.dram_tensor("wt_hh0", [128, KC * H4], F16, kind="ExternalInput").ap()
    d_wih1 = nc.dram_tensor("wt_ih1", [128, KC * H4], F16, kind="ExternalInput").ap()
    d_whh1 = nc.dram_tensor("wt_hh1", [128, KC * H4], F16, kind="ExternalInput").ap()
    d_wlin = nc.dram_tensor("wt_lin", [128, KC * D], F16, kind="ExternalInput").ap()
    d_b0t = nc.dram_tensor("b0t", [1, 16, GS], F16, kind="ExternalInput").ap()
    d_b1v = nc.dram_tensor("b1v", [1, H4], F16, kind="ExternalInput").ap()
    d_blin = nc.dram_tensor("b_lin", [D], F32, kind="ExternalInput").ap()
    d_out = nc.dram_tensor("out", [BS, nsteps, D], F32, kind="ExternalOutput").ap()
    d_h1sc = nc.dram_tensor("h1scratch", [nsteps, 128, KC, 128], F16,
                            kind="Internal").ap()

    with tile.TileContext(nc) as tc:
        build_body(nc, tc, nsteps,
                   d_zT, d_zTs, d_gamma, d_beta,
                   d_wih0, d_whh0, d_wih1, d_whh1, d_wlin,
                   d_b0t, d_b1v, d_blin, d_out, d_h1sc)
    nc.compile()
    return nc


def build_body(nc, tc, nsteps, d_zT, d_zTs, d_gamma, d_beta,
               d_wih0, d_whh0, d_wih1, d_whh1, d_wlin,
               d_b0t, d_b1v, d_blin, d_out, d_h1sc):
    # ---- whole-life SBUF (219.4 KB/partition) ---------------------------
    life = tc.alloc_tile_pool(name="life", bufs=1)
    whh0 = life.tile([128, KC, H4], F16, tag="whh0")
    wih1 = life.tile([128, KC, H4], F16, tag="wih1")
    whh1 = life.tile([128, KC, H4], F16, tag="whh1")
    x0r = life.tile([128, 16, GS], F16, tag="x0r")    # (g, q, 256) resident
    b1r = life.tile([128, 16, GS], F16, tag="b1r")    # (g, q, 256) resident
    c0 = life.tile([128, D], F16, tag="c0")
    c1 = life.tile([128, D], F16, tag="c1")
    h0T = life.tile([128, KC, 128], F16, tag="h0T")
    idf = life.tile([128, 128], F16, tag="idf")

    make_identity(nc, idf)
    # Weight arrival schedule (need-by order: staged wih0 ~asap, whh0 ~30us,
    # whh1 ~33us, wih1 mid-step-0). Chunk-pair DMAs = 16KB packets.
    # Scalar DGE queue: wih0 staging then whh0; gpsimd SWDGE queue: wih1;
    # sync queue: z/BN path + whh1 chases.
    for i in range(KC // 2):
        nc.scalar.dma_start(out=whh1[:, 2 * i:2 * i + 2, :],
                            in_=d_wih0[:, 2 * i * H4:(2 * i + 2) * H4])
    for i in range(KC // 2):
        nc.scalar.dma_start(out=whh0[:, 2 * i:2 * i + 2, :],
                            in_=d_whh0[:, 2 * i * H4:(2 * i + 2) * H4])
    b1bc = bass.AP(tensor=d_b1v.tensor, offset=d_b1v.offset,
                   ap=[[0, 128], [1, H4]])
    nc.scalar.dma_start(out=b1r, in_=b1bc)
    for i in range(KC // 2):
        nc.gpsimd.dma_start(out=wih1[:, 2 * i:2 * i + 2, :],
                            in_=d_wih1[:, 2 * i * H4:(2 * i + 2) * H4])

    # ---- INIT phase: BN -> zn^T (=h0T) ; x0_proj ; c0/c1 ----------------
    with tc.tile_pool(name="initp", bufs=4) as initp, \
         tc.tile_pool(name="small", bufs=4) as small, \
         tc.tile_pool(name="ipsum", bufs=4, space="PSUM") as ipsum:

        eps_t = small.tile([128, 1], F32, tag="eps", bufs=1)
        nc.vector.memset(eps_t, EPS)
        ones = small.tile([1, 128], F16, tag="ones", bufs=1)
        nc.vector.memset(ones, 1.0)
        # b0t aliases the first partition-row of x0r (read fully before
        # x0r's own writes; tile tracker orders the WAR hazard).
        b0t = x0r[0:1, :, :]
        nc.sync.dma_start(out=b0t, in_=d_b0t)

        # x0_proj psum: all four groups live at once (8 banks), bias first
        psg = [ipsum.tile([128, 4, GS], F32, tag="ips", name=f"ips{g}")
               for g in range(NG)]
        for g in range(NG):
            for half in range(2):
                nc.tensor.matmul(psg[g][:, 2 * half:2 * half + 2, :], ones,
                                 b0t[:, 4 * g + 2 * half:4 * g + 2 * half + 2, :],
                                 start=True, stop=False, skip_group_check=True)

        # packed per-core z shard + gamma/beta (few fat packets)
        zsp = small.tile([128, KC, BS], F32, tag="zsp", bufs=1)
        nc.sync.dma_start(out=zsp, in_=d_zTs)
        gam = small.tile([128, KC], F32, tag="gam", bufs=1)
        nc.sync.dma_start(out=gam, in_=d_gamma)
        bet = small.tile([128, KC], F32, tag="bet", bufs=1)
        nc.sync.dma_start(out=bet, in_=d_beta)

        for k in range(KC):
            # BN stats for d-chunk k from the full batch (fp16 z^T)
            zt = initp.tile([128, B], F16, tag="zt", bufs=2)
            nc.sync.dma_start(out=zt, in_=d_zT[k * 128:(k + 1) * 128, :])

            st = small.tile([128, 2, 6], F32, tag="st")
            nc.vector.bn_stats(out=st[:, 0, :], in_=zt[:, 0:512])
            nc.vector.bn_stats(out=st[:, 1, :], in_=zt[:, 512:1024])
            mv = small.tile([128, 2], F32, tag="mv")
            nc.vector.bn_aggr(out=mv, in_=st)

            sd = small.tile([128, 1], F32, tag="sd")
            nc.scalar.activation(out=sd, in_=mv[:, 1:2], func=AF.Sqrt, bias=eps_t)
            rs = small.tile([128, 1], F32, tag="rs")
            nc.vector.reciprocal(out=rs, in_=sd)
            sc = small.tile([128, 1], F32, tag="sc")
            nc.vector.tensor_mul(sc, gam[:, k:k + 1], rs)

            # zn^T chunk straight into fp16 h0T: (z - mean) * scale + beta
            nc.vector.tensor_scalar(
                out=h0T[:, k, :], in0=zsp[:, k, :],
                scalar1=mv[:, 0:1], scalar2=sc,
                op0=ALU.subtract, op1=ALU.mult)
            nc.vector.tensor_scalar_add(h0T[:, k, :], h0T[:, k, :],
                                        bet[:, k:k + 1])

            # x0_proj contribution of chunk k (wih0 staged in whh1 slot)
            for g in range(NG):
                for half in range(2):
                    nc.tensor.matmul(
                        psg[g][:, 2 * half:2 * half + 2, :], h0T[:, k, :],
                        whh1[:, k, g * D + half * 512:g * D + (half + 1) * 512],
                        start=False, stop=(k == KC - 1),
                        skip_group_check=True)
            # real whh1 pair chases the staged wih0 (WAR-ordered by tile)
            if k % 2 == 1:
                nc.sync.dma_start(out=whh1[:, k - 1:k + 1, :],
                                  in_=d_whh1[:, (k - 1) * H4:(k + 1) * H4])

        # psum -> resident x0 (fp16)
        for g in range(NG):
            nc.scalar.copy(out=x0r[:, 4 * g:4 * g + 4, :], in_=psg[g])

        # c0 = c1 = zn via PE transpose of fp16 zn^T (psum free again)
        for kk in range(KC // 2):
            pt = ipsum.tile([128, 2, 128], F16, tag="ips", name=f"tpz{kk}")
            nc.tensor.transpose(pt[:, 0, :], h0T[:, 2 * kk, :], idf)
            nc.tensor.transpose(pt[:, 1, :], h0T[:, 2 * kk + 1, :], idf)
            nc.scalar.copy(out=c0[:, kk * 256:(kk + 1) * 256], in_=pt)
        nc.vector.tensor_copy(out=c1, in_=c0)

    # ---- recurrent loop --------------------------------------------------
    h1Tp = tc.alloc_tile_pool(name="h1Tp", bufs=2)
    with tc.tile_pool(name="gates", bufs=3, space="PSUM") as gpool, \
         tc.tile_pool(name="trp", bufs=2, space="PSUM") as trpool, \
         tc.tile_pool(name="tmp", bufs=3) as tmp, \
         tc.tile_pool(name="hst", bufs=4) as hst:

        h1T = h1Tp.tile([128, KC, 128], F16, tag="h1T")
        nc.vector.tensor_copy(out=h1T, in_=h0T)

        def cell_math(g, ps, c, bias_src, hs):
            # ps[:, 0..3, :] = i, f, o, g preactivations (psum f32)
            nc.vector.tensor_add(ps, ps, bias_src[:, 4 * g:4 * g + 4, :])
            nc.scalar.activation(out=ps[:, 0:3, :], in_=ps[:, 0:3, :],
                                 func=AF.Sigmoid)
            tg = tmp.tile([128, GS], F16, tag="tp")
            nc.scalar.activation(out=tg, in_=ps[:, 3, :], func=AF.Tanh)
            csl = c[:, g * GS:(g + 1) * GS]
            nc.vector.tensor_mul(csl, csl, ps[:, 1, :])          # c *= sig(f)
            tp = tmp.tile([128, GS], F16, tag="tp")
            nc.vector.tensor_mul(tp, ps[:, 0, :], tg)            # sig(i)*tanh(g)
            nc.vector.tensor_add(csl, csl, tp)
            tc2 = tmp.tile([128, GS], F16, tag="tp")
            nc.scalar.activation(out=tc2, in_=csl, func=AF.Tanh)
            nc.vector.tensor_mul(hs, ps[:, 2, :], tc2)           # h = sig(o)*tanh(c)

        def transpose_pair(g, hs, hT):
            pt = trpool.tile([128, 256], F16, tag="tr")
            nc.tensor.transpose(pt[:, 0:128], hs[:, 0:128], idf)
            nc.tensor.transpose(pt[:, 128:256], hs[:, 128:256], idf)
            nc.scalar.copy(out=hT[:, 2 * g:2 * g + 2, :], in_=pt)

        def gates_matmul(ps, hT, w, g, first, last):
            # accumulate group g's 4 gate blocks: contiguous [128, 512] rhs
            for k in range(KC):
                for half in range(2):
                    nc.tensor.matmul(
                        ps[:, 2 * half:2 * half + 2, :], hT[:, k, :],
                        w[:, k, g * D + half * 512:g * D + (half + 1) * 512],
                        start=(first and k == 0),
                        stop=(last and k == KC - 1),
                        skip_group_check=True)

        for t in range(nsteps):
            # --- layer 0: gates0 = x0_proj + h0 @ W_hh0^T ---
            ps0 = []
            hs0 = []
            for g in range(NG):
                ps = gpool.tile([128, 4, GS], F32, tag="g")
                ps0.append(ps)
                gates_matmul(ps, h0T, whh0, g, True, True)
                hs = hst.tile([128, GS], F16, tag="h")
                hs0.append(hs)
                cell_math(g, ps, c0, x0r, hs)

            # --- layer 1 in two halves to bound PSUM use ---
            h1T_new = h1Tp.tile([128, KC, 128], F16, tag="h1T")
            for half in range(2):
                gs_ = (0, 1) if half == 0 else (2, 3)
                ps1 = {}
                for g in gs_:
                    ps = gpool.tile([128, 4, GS], F32, tag="g")
                    ps1[g] = ps
                    gates_matmul(ps, h1T, whh1, g, True, False)
                if half == 0:
                    # transpose h0 slices now (L0 math has had time to finish)
                    for g in range(NG):
                        transpose_pair(g, hs0[g], h0T)
                for g in gs_:
                    ps = ps1[g]
                    gates_matmul(ps, h0T, wih1, g, False, True)
                    hs = hst.tile([128, GS], F16, tag="h")
                    cell_math(g, ps, c1, b1r, hs)
                    transpose_pair(g, hs, h1T_new)

            nc.sync.dma_start(out=d_h1sc[t], in_=h1T_new)
            h1T = h1T_new

    # Release recurrent weights/state so the final phase can use their SBUF.
    h1Tp.release()
    life.release()

    # ---- final linear: y_t = h1_t @ W_lin^T + b_lin ---------------------
    with tc.tile_pool(name="finw", bufs=1) as finw, \
         tc.tile_pool(name="fin", bufs=3) as fin, \
         tc.tile_pool(name="ypsum", bufs=4, space="PSUM") as ypool:

        wlin = finw.tile([128, KC, D], F16, tag="wlin")
        for i in range(2):
            nc.scalar.dma_start(out=wlin[:, 4 * i:4 * i + 4, :],
                                in_=d_wlin[:, 4 * i * D:(4 * i + 4) * D])
        blin = finw.tile([128, D], F32, tag="blin")
        blin_b = bass.AP(tensor=d_blin.tensor, offset=d_blin.offset,
                         ap=[[0, 128], [1, D]])
        nc.scalar.dma_start(out=blin, in_=blin_b)

        for t in range(nsteps):
            h1in = fin.tile([128, KC, 128], F16, tag="h1in", bufs=4)
            nc.sync.dma_start(out=h1in, in_=d_h1sc[t])
            yp = ypool.tile([128, D], F32, tag="y")
            for k in range(KC):
                for n in range(2):
                    nc.tensor.matmul(
                        yp[:, n * 512:(n + 1) * 512], h1in[:, k, :],
                        wlin[:, k, n * 512:(n + 1) * 512],
                        start=(k == 0), stop=(k == KC - 1),
                        skip_group_check=True)
            ys = fin.tile([128, D], F32, tag="ys", bufs=6)
            nc.vector.tensor_add(ys, yp, blin)
            nc.scalar.dma_start(out=d_out[:, t, :], in_=ys)


_CACHE = {}


def _get_module(nsteps=BAR):
    if nsteps not in _CACHE:
        _CACHE[nsteps] = build_module(nsteps)
    return _CACHE[nsteps]


def _chunk_pack(Wt):
    """[D, N] -> [128, KC*N] with row p holding chunks k at [k*N:(k+1)*N],
    i.e. partition-contiguous DMA source (fat packets)."""
    n = Wt.shape[1]
    return np.ascontiguousarray(
        Wt.reshape(KC, 128, n).transpose(1, 0, 2).reshape(128, KC * n))


def _pack_w(W):
    """W [4D, D] torch gate rows -> chunk-packed fp16 [128, KC*4D], cols
    g*1024 + q'*256 + j with gate order q' = (i, f, o, g)."""
    W = np.asarray(W, np.float32)
    Wr = W.reshape(4, NG, GS, D)[list(PERM)]   # [q'][g][j][d]
    Wr = Wr.transpose(3, 1, 0, 2)              # [d][g][q'][j]
    return _chunk_pack(Wr.reshape(D, H4)).astype(np.float16)


def _pack_b(b):
    b = np.asarray(b, np.float32)
    br = b.reshape(4, NG, GS)[list(PERM)]      # [q'][g][j]
    return np.ascontiguousarray(br.transpose(1, 0, 2))  # [g][q'][j]


def prep_inputs(z, bn_gamma, bn_beta, W_ih0, W_hh0, b_ih0, b_hh0,
                W_ih1, W_hh1, b_ih1, b_hh1, W_lin, b_lin):
    z = np.asarray(z, np.float32)
    zT = np.ascontiguousarray(z.T)
    b0 = _pack_b(np.asarray(b_ih0, np.float32) + np.asarray(b_hh0, np.float32))
    b1 = _pack_b(np.asarray(b_ih1, np.float32) + np.asarray(b_hh1, np.float32))
    common = {
        "zT": zT.astype(np.float16),
        "gamma": np.ascontiguousarray(
            np.asarray(bn_gamma, np.float32).reshape(KC, 128).T),
        "beta": np.ascontiguousarray(
            np.asarray(bn_beta, np.float32).reshape(KC, 128).T),
        "wt_ih0": _pack_w(W_ih0),
        "wt_hh0": _pack_w(W_hh0),
        "wt_ih1": _pack_w(W_ih1),
        "wt_hh1": _pack_w(W_hh1),
        "wt_lin": _chunk_pack(np.asarray(W_lin, np.float32).T)
                    .astype(np.float16),
        "b0t": b0.reshape(1, 16, GS).astype(np.float16),
        "b1v": b1.reshape(1, H4).astype(np.float16),
        "b_lin": np.asarray(b_lin, np.float32),
    }
    in_maps = []
    for c in range(NCORES):
        m = dict(common)
        m["zTs"] = _chunk_pack(zT[:, c * BS:(c + 1) * BS]).astype(np.float32)
        in_maps.append(m)
    return in_maps


def kernel(**inputs):
    nc = _get_module()
    in_maps = prep_inputs(**inputs)
    res = bass_utils.run_bass_kernel_spmd(nc, in_maps, core_ids=list(range(NCORES)))
    out = np.concatenate([res.results[c]["out"] for c in range(NCORES)], axis=0)
    return out.astype(np.float32)


# revision 31
# speedup vs baseline: 1.0476x; 1.0039x over previous
"""Trainium2 Bass kernel: BatchNorm -> 2-layer LSTM (32 steps, constant layer-0
input) -> Linear, data-parallel over batch across 8 NeuronCores.

v2 layout strategy (per core, batch shard = 128 rows):
  - All matmuls: out[b, j] = lhsT.T @ rhs with lhsT = h^T chunks [128d, 128b]
    (stationary, fp16) and rhs = W^T chunks [128d, N] (moving, fp16),
    fp32 PSUM accumulation over 8 contraction chunks of d=1024.
  - Weights are host-packed fp16 in GROUP-MAJOR column order with gate
    order (i, f, o, g): col = g*1024 + q*256 + j.  Each gates matmul is
    then a contiguous [128, 512] rhs slice (one PSUM bank per instr,
    2 instrs per (group, k-chunk)) -- 2x fewer PE instructions.
  - x0_proj (+b0) and the L1 bias are RESIDENT in SBUF: zero per-step
    HBM traffic except the h1^T scratch store for the deferred linear.
  - wih0 (init-only) is staged through whh1's SBUF slot chunk-by-chunk;
    whh1's real load is issued right behind it.  b0t aliases x0r's rows.
  - Gate activations run in-place on PSUM (sigmoid over the contiguous
    i,f,o block, one instr), cell state c kept in fp16.
"""

import os
import sys

sys.path.insert(0, "/opt/trn_rl_repo")

import numpy as np

import concourse.bass as bass
import concourse.bacc as bacc
import concourse.tile as tile
import concourse.mybir as mybir
from concourse import bass_utils
from concourse.masks import make_identity

F16 = mybir.dt.float16
F32 = mybir.dt.float32
AF = mybir.ActivationFunctionType
ALU = mybir.AluOpType

B = 1024          # batch
D = 1024          # hidden = input size
H4 = 4 * D        # gate width
BAR = int(os.environ.get("KERNEL_NSTEPS", "32"))
NCORES = 8
BS = B // 8       # batch shard per core
EPS = 1e-5
KC = D // 128     # contraction chunks (8)
NG = 4            # gate column groups per layer
GS = D // NG      # group size in hidden cols (256)
PERM = (0, 1, 3, 2)  # gate order i,f,o,g (torch order is i,f,g,o)


def build_module(nsteps=BAR):
    nc = bacc.Bacc(
        "TRN2",
        target_bir_lowering=False,
        debug=False,
        enable_asserts=False,
        num_devices=NCORES,
        dynamic_dma_scratch_size=512,
    )

    # ---- DRAM I/O -------------------------------------------------------
    # Weights/z are host-packed partition-contiguous ([128, k*...]) so DMA
    # packets are 8-16KB instead of sub-KB (DMA is latency-bound per packet).
    d_zT = nc.dram_tensor("zT", [D, B], F16, kind="ExternalInput").ap()
    d_zTs = nc.dram_tensor("zTs", [128, KC * BS], F32, kind="ExternalInput").ap()
    d_gamma = nc.dram_tensor("gamma", [128, KC], F32, kind="ExternalInput").ap()
    d_beta = nc.dram_tensor("beta", [128, KC], F32, kind="ExternalInput").ap()
    d_wih0 = nc.dram_tensor("wt_ih0", [128, KC * H4], F16, kind="ExternalInput").ap()
    d_whh0 = nc.dram_tensor("wt_hh0", [128, KC * H4], F16, kind="ExternalInput").ap()
    d_wih1 = nc.dram_tensor("wt_ih1", [128, KC * H4], F16, kind="ExternalInput").ap()
    d_whh1 = nc.dram_tensor("wt_hh1", [128, KC * H4], F16, kind="ExternalInput").ap()
    d_wlin = nc.dram_tensor("wt_lin", [128, KC * D], F16, kind="ExternalInput").ap()
    d_b0t = nc.dram_tensor("b0t", [1, 16, GS], F16, kind="ExternalInput").ap()
    d_b1v = nc.dram_tensor("b1v", [1, H4], F16, kind="ExternalInput").ap()
    d_blin = nc.dram_tensor("b_lin", [D], F32, kind="ExternalInput").ap()
    d_out = nc.dram_tensor("out", [BS, nsteps, D], F32, kind="ExternalOutput").ap()
    d_h1sc = nc.dram_tensor("h1scratch", [nsteps, 128, KC, 128], F16,
                            kind="Internal").ap()

    with tile.TileContext(nc) as tc:
        build_body(nc, tc, nsteps,
                   d_zT, d_zTs, d_gamma, d_beta,
                   d_wih0, d_whh0, d_wih1, d_whh1, d_wlin,
                   d_b0t, d_b1v, d_blin, d_out, d_h1sc)
    nc.compile()
    return nc


def build_body(nc, tc, nsteps, d_zT, d_zTs, d_gamma, d_beta,
               d_wih0, d_whh0, d_wih1, d_whh1, d_wlin,
               d_b0t, d_b1v, d_blin, d_out, d_h1sc):
    # ---- whole-life SBUF (219.4 KB/partition) ---------------------------
    life = tc.alloc_tile_pool(name="life", bufs=1)
    whh0 = life.tile([128, KC, H4], F16, tag="whh0")
    wih1 = life.tile([128, KC, H4], F16, tag="wih1")
    whh1 = life.tile([128, KC, H4], F16, tag="whh1")
    x0r = life.tile([128, 16, GS], F16, tag="x0r")    # (g, q, 256) resident
    b1r = life.tile([128, 16, GS], F16, tag="b1r")    # (g, q, 256) resident
    c0 = life.tile([128, D], F16, tag="c0")
    c1 = life.tile([128, D], F16, tag="c1")
    h0T = life.tile([128, KC, 128], F16, tag="h0T")
    idf = life.tile([128, 128], F16, tag="idf")

    make_identity(nc, idf)
    # Weight arrival schedule, balanced across the 3 DGE queues by need-by
    # time (DMA is bandwidth-bound ~400GB/s aggregate).  Chunk-pair DMAs =
    # 16KB packets.  scalar: wih0 staging (asap) + b1; sync: z/BN path then
    # whh0; gpsimd: whh1 chases then wih1 (last-needed).
    for i in range(KC // 2):
        nc.scalar.dma_start(out=whh1[:, 2 * i:2 * i + 2, :],
                            in_=d_wih0[:, 2 * i * H4:(2 * i + 2) * H4])
    b1bc = bass.AP(tensor=d_b1v.tensor, offset=d_b1v.offset,
                   ap=[[0, 128], [1, H4]])
    nc.scalar.dma_start(out=b1r, in_=b1bc)

    # ---- INIT phase: BN -> zn^T (=h0T) ; x0_proj ; c0/c1 ----------------
    with tc.tile_pool(name="initp", bufs=4) as initp, \
         tc.tile_pool(name="small", bufs=4) as small, \
         tc.tile_pool(name="ipsum", bufs=4, space="PSUM") as ipsum:

        eps_t = small.tile([128, 1], F32, tag="eps", bufs=1)
        nc.vector.memset(eps_t, EPS)
        ones = small.tile([1, 128], F16, tag="ones", bufs=1)
        nc.vector.memset(ones, 1.0)
        # b0t aliases the first partition-row of x0r (read fully before
        # x0r's own writes; tile tracker orders the WAR hazard).
        b0t = x0r[0:1, :, :]
        nc.sync.dma_start(out=b0t, in_=d_b0t)

        # x0_proj psum: all four groups live at once (8 banks), bias first
        psg = [ipsum.tile([128, 4, GS], F32, tag="ips", name=f"ips{g}")
               for g in range(NG)]
        for g in range(NG):
            for half in range(2):
                nc.tensor.matmul(psg[g][:, 2 * half:2 * half + 2, :], ones,
                                 b0t[:, 4 * g + 2 * half:4 * g + 2 * half + 2, :],
                                 start=True, stop=False, skip_group_check=True)

        # packed per-core z shard + gamma/beta (few fat packets)
        zsp = small.tile([128, KC, BS], F32, tag="zsp", bufs=1)
        nc.sync.dma_start(out=zsp, in_=d_zTs)
        gam = small.tile([128, KC], F32, tag="gam", bufs=1)
        nc.sync.dma_start(out=gam, in_=d_gamma)
        bet = small.tile([128, KC], F32, tag="bet", bufs=1)
        nc.sync.dma_start(out=bet, in_=d_beta)

        for k in range(KC):
            # BN stats for d-chunk k from the full batch (fp16 z^T)
            zt = initp.tile([128, B], F16, tag="zt", bufs=2)
            nc.sync.dma_start(out=zt, in_=d_zT[k * 128:(k + 1) * 128, :])

            st = small.tile([128, 2, 6], F32, tag="st")
            nc.vector.bn_stats(out=st[:, 0, :], in_=zt[:, 0:512])
            nc.vector.bn_stats(out=st[:, 1, :], in_=zt[:, 512:1024])
            mv = small.tile([128, 2], F32, tag="mv")
            nc.vector.bn_aggr(out=mv, in_=st)

            sd = small.tile([128, 1], F32, tag="sd")
            nc.scalar.activation(out=sd, in_=mv[:, 1:2], func=AF.Sqrt, bias=eps_t)
            rs = small.tile([128, 1], F32, tag="rs")
            nc.vector.reciprocal(out=rs, in_=sd)
            sc = small.tile([128, 1], F32, tag="sc")
            nc.vector.tensor_mul(sc, gam[:, k:k + 1], rs)

            # zn^T chunk straight into fp16 h0T: (z - mean) * scale + beta
            nc.vector.tensor_scalar(
                out=h0T[:, k, :], in0=zsp[:, k, :],
                scalar1=mv[:, 0:1], scalar2=sc,
                op0=ALU.subtract, op1=ALU.mult)
            nc.vector.tensor_scalar_add(h0T[:, k, :], h0T[:, k, :],
                                        bet[:, k:k + 1])

            # x0_proj contribution of chunk k (wih0 staged in whh1 slot)
            for g in range(NG):
                for half in range(2):
                    nc.tensor.matmul(
                        psg[g][:, 2 * half:2 * half + 2, :], h0T[:, k, :],
                        whh1[:, k, g * D + half * 512:g * D + (half + 1) * 512],
                        start=False, stop=(k == KC - 1),
                        skip_group_check=True)
            # real whh1 pair chases the staged wih0 (WAR-ordered by tile)
            if k % 2 == 1:
                nc.sync.dma_start(out=whh1[:, k - 1:k + 1, :],
                                  in_=d_whh1[:, (k - 1) * H4:(k + 1) * H4])

        # whh0 behind the z path on sync; wih1 behind the chases on gpsimd
        for i in range(KC // 2):
            nc.sync.dma_start(out=whh0[:, 2 * i:2 * i + 2, :],
                              in_=d_whh0[:, 2 * i * H4:(2 * i + 2) * H4])
        for i in range(KC // 2):
            nc.gpsimd.dma_start(out=wih1[:, 2 * i:2 * i + 2, :],
                                in_=d_wih1[:, 2 * i * H4:(2 * i + 2) * H4])

        # psum -> resident x0 (fp16)
        for g in range(NG):
            nc.scalar.copy(out=x0r[:, 4 * g:4 * g + 4, :], in_=psg[g])

        # c0 = c1 = zn via PE transpose of fp16 zn^T (psum free again)
        for kk in range(KC // 2):
            pt = ipsum.tile([128, 2, 128], F16, tag="ips", name=f"tpz{kk}")
            nc.tensor.transpose(pt[:, 0, :], h0T[:, 2 * kk, :], idf)
            nc.tensor.transpose(pt[:, 1, :], h0T[:, 2 * kk + 1, :], idf)
            nc.scalar.copy(out=c0[:, kk * 256:(kk + 1) * 256], in_=pt)
        nc.vector.tensor_copy(out=c1, in_=c0)

    # ---- recurrent loop --------------------------------------------------
    h1Tp = tc.alloc_tile_pool(name="h1Tp", bufs=2)
    with tc.tile_pool(name="gates", bufs=3, space="PSUM") as gpool, \
         tc.tile_pool(name="trp", bufs=2, space="PSUM") as trpool, \
         tc.tile_pool(name="tmp", bufs=3) as tmp, \
         tc.tile_pool(name="hst", bufs=4) as hst:

        h1T = h1Tp.tile([128, KC, 128], F16, tag="h1T")
        nc.vector.tensor_copy(out=h1T, in_=h0T)

        def cell_math(g, ps, c, bias_src, hs):
            # ps[:, 0..3, :] = i, f, o, g preactivations (psum f32)
            nc.vector.tensor_add(ps, ps, bias_src[:, 4 * g:4 * g + 4, :])
            nc.scalar.activation(out=ps[:, 0:3, :], in_=ps[:, 0:3, :],
                                 func=AF.Sigmoid)
            tg = tmp.tile([128, GS], F16, tag="tp")
            nc.scalar.activation(out=tg, in_=ps[:, 3, :], func=AF.Tanh)
            csl = c[:, g * GS:(g + 1) * GS]
            nc.vector.tensor_mul(csl, csl, ps[:, 1, :])          # c *= sig(f)
            tp = tmp.tile([128, GS], F16, tag="tp")
            nc.vector.tensor_mul(tp, ps[:, 0, :], tg)            # sig(i)*tanh(g)
            nc.vector.tensor_add(csl, csl, tp)
            tc2 = tmp.tile([128, GS], F16, tag="tp")
            nc.scalar.activation(out=tc2, in_=csl, func=AF.Tanh)
            nc.vector.tensor_mul(hs, ps[:, 2, :], tc2)           # h = sig(o)*tanh(c)

        def transpose_pair(g, hs, hT):
            pt = trpool.tile([128, 256], F16, tag="tr")
            nc.tensor.transpose(pt[:, 0:128], hs[:, 0:128], idf)
            nc.tensor.transpose(pt[:, 128:256], hs[:, 128:256], idf)
            nc.scalar.copy(out=hT[:, 2 * g:2 * g + 2, :], in_=pt)

        def gates_matmul(ps, hT, w, g, first, last):
            # accumulate group g's 4 gate blocks: contiguous [128, 512] rhs
            for k in range(KC):
                for half in range(2):
                    nc.tensor.matmul(
                        ps[:, 2 * half:2 * half + 2, :], hT[:, k, :],
                        w[:, k, g * D + half * 512:g * D + (half + 1) * 512],
                        start=(first and k == 0),
                        stop=(last and k == KC - 1),
                        skip_group_check=True)

        for t in range(nsteps):
            # --- layer 0: gates0 = x0_proj + h0 @ W_hh0^T ---
            ps0 = []
            hs0 = []
            for g in range(NG):
                ps = gpool.tile([128, 4, GS], F32, tag="g")
                ps0.append(ps)
                gates_matmul(ps, h0T, whh0, g, True, True)
                hs = hst.tile([128, GS], F16, tag="h")
                hs0.append(hs)
                cell_math(g, ps, c0, x0r, hs)

            # --- layer 1 in two halves to bound PSUM use ---
            h1T_new = h1Tp.tile([128, KC, 128], F16, tag="h1T")
            for half in range(2):
                gs_ = (0, 1) if half == 0 else (2, 3)
                ps1 = {}
                for g in gs_:
                    ps = gpool.tile([128, 4, GS], F32, tag="g")
                    ps1[g] = ps
                    gates_matmul(ps, h1T, whh1, g, True, False)
                if half == 0:
                    # transpose h0 slices now (L0 math has had time to finish)
                    for g in range(NG):
                        transpose_pair(g, hs0[g], h0T)
                for g in gs_:
                    ps = ps1[g]
                    gates_matmul(ps, h0T, wih1, g, False, True)
                    hs = hst.tile([128, GS], F16, tag="h")
                    cell_math(g, ps, c1, b1r, hs)
                    transpose_pair(g, hs, h1T_new)

            nc.sync.dma_start(out=d_h1sc[t], in_=h1T_new)
            h1T = h1T_new

    # Release recurrent weights/state so the final phase can use their SBUF.
    h1Tp.release()
    life.release()

    # ---- final linear: y_t = h1_t @ W_lin^T + b_lin ---------------------
    with tc.tile_pool(name="finw", bufs=1) as finw, \
         tc.tile_pool(name="fin", bufs=3) as fin, \
         tc.tile_pool(name="ypsum", bufs=4, space="PSUM") as ypool:

        wlin = finw.tile([128, KC, D], F16, tag="wlin")
        for i in range(2):
            nc.scalar.dma_start(out=wlin[:, 4 * i:4 * i + 4, :],
                                in_=d_wlin[:, 4 * i * D:(4 * i + 4) * D])
        blin = finw.tile([128, D], F32, tag="blin")
        blin_b = bass.AP(tensor=d_blin.tensor, offset=d_blin.offset,
                         ap=[[0, 128], [1, D]])
        nc.scalar.dma_start(out=blin, in_=blin_b)

        for t in range(nsteps):
            h1in = fin.tile([128, KC, 128], F16, tag="h1in", bufs=4)
            nc.sync.dma_start(out=h1in, in_=d_h1sc[t])
            yp = ypool.tile([128, D], F32, tag="y")
            for k in range(KC):
                for n in range(2):
                    nc.tensor.matmul(
                        yp[:, n * 512:(n + 1) * 512], h1in[:, k, :],
                        wlin[:, k, n * 512:(n + 1) * 512],
                        start=(k == 0), stop=(k == KC - 1),
                        skip_group_check=True)
            ys = fin.tile([128, D], F32, tag="ys", bufs=6)
            nc.vector.tensor_add(ys, yp, blin)
            nc.scalar.dma_start(out=d_out[:, t, :], in_=ys)


_CACHE = {}


def _get_module(nsteps=BAR):
    if nsteps not in _CACHE:
        _CACHE[nsteps] = build_module(nsteps)
    return _CACHE[nsteps]


def _chunk_pack(Wt):
    """[D, N] -> [128, KC*N] with row p holding chunks k at [k*N:(k+1)*N],
    i.e. partition-contiguous DMA source (fat packets)."""
    n = Wt.shape[1]
    return np.ascontiguousarray(
        Wt.reshape(KC, 128, n).transpose(1, 0, 2).reshape(128, KC * n))


def _pack_w(W):
    """W [4D, D] torch gate rows -> chunk-packed fp16 [128, KC*4D], cols
    g*1024 + q'*256 + j with gate order q' = (i, f, o, g)."""
    W = np.asarray(W, np.float32)
    Wr = W.reshape(4, NG, GS, D)[list(PERM)]   # [q'][g][j][d]
    Wr = Wr.transpose(3, 1, 0, 2)              # [d][g][q'][j]
    return _chunk_pack(Wr.reshape(D, H4)).astype(np.float16)


def _pack_b(b):
    b = np.asarray(b, np.float32)
    br = b.reshape(4, NG, GS)[list(PERM)]      # [q'][g][j]
    return np.ascontiguousarray(br.transpose(1, 0, 2))  # [g][q'][j]


def prep_inputs(z, bn_gamma, bn_beta, W_ih0, W_hh0, b_ih0, b_hh0,
                W_ih1, W_hh1, b_ih1, b_hh1, W_lin, b_lin):
    z = np.asarray(z, np.float32)
    zT = np.ascontiguousarray(z.T)
    b0 = _pack_b(np.asarray(b_ih0, np.float32) + np.asarray(b_hh0, np.float32))
    b1 = _pack_b(np.asarray(b_ih1, np.float32) + np.asarray(b_hh1, np.float32))
    common = {
        "zT": zT.astype(np.float16),
        "gamma": np.ascontiguousarray(
            np.asarray(bn_gamma, np.float32).reshape(KC, 128).T),
        "beta": np.ascontiguousarray(
            np.asarray(bn_beta, np.float32).reshape(KC, 128).T),
        "wt_ih0": _pack_w(W_ih0),
        "wt_hh0": _pack_w(W_hh0),
        "wt_ih1": _pack_w(W_ih1),
        "wt_hh1": _pack_w(W_hh1),
        "wt_lin": _chunk_pack(np.asarray(W_lin, np.float32).T)
                    .astype(np.float16),
        "b0t": b0.reshape(1, 16, GS).astype(np.float16),
        "b1v": b1.reshape(1, H4).astype(np.float16),
        "b_lin": np.asarray(b_lin, np.float32),
    }
    in_maps = []
    for c in range(NCORES):
        m = dict(common)
        m["zTs"] = _chunk_pack(zT[:, c * BS:(c + 1) * BS]).astype(np.float32)
        in_maps.append(m)
    return in_maps


def kernel(**inputs):
    nc = _get_module()
    in_maps = prep_inputs(**inputs)
    res = bass_utils.run_bass_kernel_spmd(nc, in_maps, core_ids=list(range(NCORES)))
    out = np.concatenate([res.results[c]["out"] for c in range(NCORES)], axis=0)
    return out.astype(np.float32)
